# revision 43
# baseline (speedup 1.0000x reference)
"""Multi-level ROI Align (FPN pooler, 4 levels summed) on 8 Trainium2 cores.

v3.3: descriptor- and byte-minimized fp16 design. Shard ROIs across cores
(core k: batch k//4, 128 ROIs as 64 pairs). Host computes all gather
indices and bilinear weights from `boxes`; device does HBM strip-gathers
(dma_gather, one descriptor per multi-pixel fp16 strip, 4 SWDGE queues)
+ fp16 matmuls accumulating both ROIs of a pair into one PSUM tile
[98, 256] (bins 0-48 = ROI a, 49-97 = ROI b), evicted to DRAM bin-major
[98, 64pair, 256] fp32; host transposes.

Per pair-of-ROIs (62+ matmuls, ~616 gather descriptors):
  L0 (200x200, s=.25):  2x196 strips of 8px (even-aligned, idx=flat>>1),
      one per (ysample, rowsel, xbin); rank-1 weights (fixed one-hot bin
      pattern x per-strip scalar, one broadcast DVE op) -> 32 matmuls
  L1 (100x100, s=.125): 2x112 strips of 9px, one per (ysample, rowsel,
      xbin-pair); host-baked dense lhsT (DMA'd per block) -> 18 mm
  L2 (50x50, s=.0625):  NO gather -- full map SBUF-resident, padded to
      [50,64] rows (25 chunks); weights built on device from separable
      per-ROI wy[25ch,7]/wx[7] tables via 2 DVE ops -> 25 mm
  L3 (25x25, s=.03125): NO gather -- full map SBUF-resident (5 chunks);
      host-baked dense separable lhsT -> 5 mm
"""
import sys
import numpy as np

sys.path.insert(0, '/opt/trn_rl_repo')

POOLED = 7
SAMP = 2
NBIN = 49
C = 256
IMG = 800.0

NROI_CORE = 128
NPAIR = 64

# per-level geometry
L0 = dict(H=200, W=200, scale=0.25)
L1 = dict(H=100, W=100, scale=0.125)
L2 = dict(H=50, W=50, scale=0.0625)
L3 = dict(H=25, W=25, scale=0.03125)

W0, W1 = 8, 9                          # strip widths (px)
NS0, NS1 = 196, 112                    # strips per ROI
N0, N1 = 400, 224                      # padded pair nidx (%16, incl -1 tail)
CH0, CH1 = 4, 2                        # chunks per pair
CH2 = 25                               # L2 chunks: [50, 64]-padded map
CH3 = 5                                # L3 full-map chunks (625px -> 5x128)
NT1, NT3 = CH1 * W1, CH3               # dense lhsT tiles (L1, L3)
NTD = NT1 + NT3                        # 18 + 5 = 23
NBIN2 = 2 * NBIN                       # 98
WY2 = CH2 * POOLED                     # 175 wy2 cols per ROI
PAIR_WROW = NTD * NBIN2 + 2 * (WY2 + POOLED)   # 2618 wd els per pair
WYO = NTD * NBIN2                      # wy2/wx2 table offset in wd row
NBLK = NPAIR // 2                      # 2-pair blocks

F0_ROWS = 40004
F1_ROWS = 10000
F2_ROWS = 2560
F3_ROWS = 640

# cst fp16 column layout: L0 pattern [4ch, 98] then L0 scalars
PAT0_COLS = CH0 * NBIN2                        # 392
SCAL0_PER_PAIR = CH0 * W0                      # 32
CST_COLS = PAT0_COLS + NPAIR * SCAL0_PER_PAIR  # 392 + 2048

# idx int16 layout per pair: [L0 400][L1 224] / 16
PAIR_ICOLS = (N0 + N1) // 16                   # 39
IDX_COLS = NPAIR * PAIR_ICOLS

_MODULE_CACHE = {}


def _sample_meta(boxes_b, H, W, scale):
    """Per-ROI sample geometry in fp32, matching reference op order.
    boxes_b: [N, 4] fp32. Returns dict of [N,7,2] arrays."""
    f = np.float32
    b = boxes_b.astype(np.float32)
    x1 = b[:, 0] * f(scale)
    y1 = b[:, 1] * f(scale)
    x2 = b[:, 2] * f(scale)
    y2 = b[:, 3] * f(scale)
    rw = np.maximum(x2 - x1, f(1.0))
    rh = np.maximum(y2 - y1, f(1.0))
    bw = rw / f(POOLED)
    bh = rh / f(POOLED)
    g = (np.arange(POOLED, dtype=np.float32)[:, None]
         + (np.arange(SAMP, dtype=np.float32)[None, :] + f(0.5)) / f(SAMP))
    y = y1[:, None, None] + g[None] * bh[:, None, None]   # [N,7,2]
    x = x1[:, None, None] + g[None] * bw[:, None, None]
    masky = ((y >= f(-1.0)) & (y <= f(H))).astype(np.float32)
    maskx = ((x >= f(-1.0)) & (x <= f(W))).astype(np.float32)
    yc = np.clip(y, f(0.0), f(H - 1))
    xc = np.clip(x, f(0.0), f(W - 1))
    yl = np.floor(yc).astype(np.int64)
    xl = np.floor(xc).astype(np.int64)
    yh = np.minimum(yl + 1, H - 1)
    xh = np.minimum(xl + 1, W - 1)
    ly = (yc - yl.astype(np.float32)).astype(np.float32)
    lx = (xc - xl.astype(np.float32)).astype(np.float32)
    hy = (f(1.0) - ly).astype(np.float32)
    hx = (f(1.0) - lx).astype(np.float32)
    return dict(yl=yl, yh=yh, xl=xl, xh=xh, ly=ly, lx=lx, hy=hy, hx=hx,
                masky=masky, maskx=maskx, x=x, y=y)


def _strip_grid(meta, W, bins_per_strip, width, parity):
    """Build per-ROI strip indices and slot weights.

    Strips: (rowsel 2) x (ysample 14) x (xgroup ceil(7/b)).
    Returns idx [N, NS] int64 (pixel index of strip start, or start>>1 if
    parity), wslot [N, NS, width] fp32 (bilinear x-weights x y-weight x
    mask x 0.25), and bins [NS_xgroups arrays] for pattern building is
    implicit: each strip's samples' bins vary within the group -- handled
    by caller via per-sample info: also returns sample slot/bin arrays:
    contrib = (slotpos [N, NS, 7bins?..]) -- instead we return dense
    per-strip per-slot per-bin weights only when needed. For rank-1 (b=1)
    wslot is enough (all slots -> the strip's single bin).
    """
    N = meta['yl'].shape[0]
    f = np.float32
    ngrp = -(-POOLED // bins_per_strip)           # x-groups per row
    NS = 2 * 14 * ngrp
    # rows/yweights: [N, 2, 14]
    rows = np.stack([meta['yl'], meta['yh']], axis=1).reshape(N, 2, 14)
    wy = (np.stack([meta['hy'], meta['ly']], axis=1)
          * meta['masky'][:, None]).reshape(N, 2, 14).astype(np.float32)
    # x corners per sample: [N, 7, 2]
    xl, xh = meta['xl'], meta['xh']
    wxl = (meta['hx'] * meta['maskx']).astype(np.float32)
    wxh = (meta['lx'] * meta['maskx']).astype(np.float32)
    # group starts: min xl over samples in group -> [N, ngrp]
    xs = np.empty((N, ngrp), np.int64)
    for gi in range(ngrp):
        b0, b1 = gi * bins_per_strip, min((gi + 1) * bins_per_strip, POOLED)
        xs[:, gi] = xl[:, b0:b1, :].reshape(N, -1).min(axis=1)
    if parity:
        xs &= ~1
    xs = np.clip(xs, 0, W - width)
    # slot weights [N, ngrp, width] per (bin-in-group, sx): scatter
    wslot = np.zeros((N, ngrp, width, POOLED), np.float32)  # per-bin slots
    ridx = np.arange(N)[:, None, None]
    for gi in range(ngrp):
        b0, b1 = gi * bins_per_strip, min((gi + 1) * bins_per_strip, POOLED)
        for bx in range(b0, b1):
            for sx in range(SAMP):
                ol = xl[:, bx, sx] - xs[:, gi]
                oh = xh[:, bx, sx] - xs[:, gi]
                np.add.at(wslot, (np.arange(N), gi, ol, bx), wxl[:, bx, sx])
                np.add.at(wslot, (np.arange(N), gi, oh, bx), wxh[:, bx, sx])
    # combine with y: strips ordered (ysample, rowsel, xgroup) so the
    # yl/yh descriptors of one sample hit adjacent DRAM rows back-to-back
    idx = (rows[:, :, :, None] * W + xs[:, None, None, :])
    if parity:
        idx >>= 1
    idx = idx.transpose(0, 2, 1, 3).reshape(N, NS)       # [N,14,2,ngrp]
    w = (wy[:, :, :, None, None, None]
         * wslot[:, None, None, :, :, :] * f(0.25))      # [N,2,14,g,w,7]
    w = w.transpose(0, 2, 1, 3, 4, 5).reshape(N, NS, width, POOLED)
    return idx, w


def _sep_tables(meta, HW):
    """Separable bilinear weight tables WY/WX [N, HW, 7] fp32."""
    N = meta['yl'].shape[0]
    f = np.float32
    WY = np.zeros((N, HW, POOLED), np.float32)
    WX = np.zeros((N, HW, POOLED), np.float32)
    ridx = np.arange(N)[:, None, None]
    pidx = np.broadcast_to(np.arange(POOLED)[None, :, None], (N, POOLED, SAMP))
    np.add.at(WY, (ridx, meta['yl'], pidx),
              (f(0.5) * meta['hy'] * meta['masky']).astype(np.float32))
    np.add.at(WY, (ridx, meta['yh'], pidx),
              (f(0.5) * meta['ly'] * meta['masky']).astype(np.float32))
    np.add.at(WX, (ridx, meta['xl'], pidx),
              (f(0.5) * meta['hx'] * meta['maskx']).astype(np.float32))
    np.add.at(WX, (ridx, meta['xh'], pidx),
              (f(0.5) * meta['lx'] * meta['maskx']).astype(np.float32))
    return WY, WX


def _build_dense_full(meta, HW, nch):
    """Dense separable weights over the FULL HWxHW map (features live in
    SBUF on device). Returns lhsT [N, nch*128, 49] fp32 (px zero-padded)."""
    N = meta['yl'].shape[0]
    WY, WX = _sep_tables(meta, HW)
    lhsT = np.einsum('nap,nbq->nabpq', WY, WX).reshape(N, HW * HW, NBIN)
    out = np.zeros((N, nch * 128, NBIN), np.float32)
    out[:, :HW * HW] = lhsT
    return out


def _pack_idx(idx_flat):
    """[n] int -> [128, n//16] int16: wrap 16 partitions, replicate 8x."""
    n = idx_flat.shape[0]
    arr = idx_flat.reshape(n // 16, 16).T            # [16, cols]
    arr = np.broadcast_to(arr[None], (8, 16, n // 16)).reshape(128, n // 16)
    return arr.astype(np.int16)


def _l0_pattern():
    """Fixed one-hot [128, CH0, 98] fp16: J = c*128+p -> bin.
    j order within an ROI: (ysample, rowsel, xbin)."""
    pat = np.zeros((CH0, 128, NBIN2), np.float16)
    for J in range(2 * NS0):
        half, j = J // NS0, J % NS0
        bx = j % POOLED
        t = j // (2 * POOLED)
        py = t // 2
        pat[J // 128, J % 128, py * 7 + bx + half * NBIN] = 1.0
    return pat.transpose(1, 0, 2)                    # [128, CH0, 98]


def _strip_scatter(wd, w, half, ns, ngrp, width, t_base, chunks_w):
    """Scatter per-ROI strip weights [NROI_CORE, ns, width, 7] into wd
    [NPAIR, 128, NTD, 98]. half: 0/1 (roi parity within pair)."""
    jj = np.arange(ns)
    J = half * ns + jj
    ch, pp = J // 128, J % 128
    py = (jj // (2 * ngrp)) // 2
    t = t_base + ch[:, None] * chunks_w + np.arange(width)[None, :]  # [ns, width]
    bn = py[:, None] * 7 + np.arange(POOLED)[None, :] + half * NBIN  # [ns, 7]
    rois = np.arange(half, NROI_CORE, 2)
    pair_i = np.broadcast_to((rois // 2)[:, None, None, None],
                             (NPAIR, ns, width, POOLED))
    pp_i = np.broadcast_to(pp[None, :, None, None], pair_i.shape)
    t_i = np.broadcast_to(t[None, :, :, None], pair_i.shape)
    bn_i = np.broadcast_to(bn[None, :, None, :], pair_i.shape)
    np.add.at(wd, (pair_i, pp_i, t_i, bn_i), w[rois])


def _host_prepare(x0, x1, x2, x3, boxes):
    """Build all per-core input tensors. Returns list of 8 dicts."""
    B = boxes.shape[0]
    feats = []
    for arr, lv, rows in ((x0, L0, F0_ROWS), (x1, L1, F1_ROWS),
                          (x2, L2, F2_ROWS), (x3, L3, F3_ROWS)):
        f = np.zeros((B, rows, C), np.float16)
        hw = lv['H'] * lv['W']
        f[:, :hw] = np.ascontiguousarray(
            np.transpose(np.asarray(arr, np.float32), (0, 2, 3, 1))
        ).reshape(B, hw, C).astype(np.float16)
        feats.append(f)
    # f2 padded to [50, 64] rows so px = y*64+x -> y = 2ch + p//64, x = p%64
    f2pad = np.zeros((B, 50, 64, C), np.float16)
    f2pad[:, :, :50] = feats[2][:, :2500].reshape(B, 50, 50, C)
    f2s = np.ascontiguousarray(
        f2pad.reshape(B, CH2, 128, C).transpose(0, 2, 1, 3))
    # f3 for SBUF residency: [128, CH3, C], px = ch*128 + p
    f3s = np.ascontiguousarray(
        feats[3].reshape(B, CH3, 128, C).transpose(0, 2, 1, 3))

    pat0 = _l0_pattern()

    per_batch = []
    for b in range(B):
        bb = np.asarray(boxes[b], np.float32)
        m0 = _sample_meta(bb, L0['H'], L0['W'], L0['scale'])
        m1 = _sample_meta(bb, L1['H'], L1['W'], L1['scale'])
        idx0, w0 = _strip_grid(m0, L0['W'], 1, W0, parity=True)   # [N,196],[N,196,8,7]
        idx1, w1 = _strip_grid(m1, L1['W'], 2, W1, parity=False)  # [N,112],[N,112,9,7]
        per_batch.append((idx0, w0, idx1, w1))

    in_maps = []
    for k in range(8):
        b = k // 4
        s = (k % 4) * NROI_CORE
        idx0, w0, idx1, w1 = per_batch[b]
        sl = slice(s, s + NROI_CORE)
        bb = np.asarray(boxes[b][sl], np.float32)
        m2 = _sample_meta(bb, L2['H'], L2['W'], L2['scale'])
        m3 = _sample_meta(bb, L3['H'], L3['W'], L3['scale'])
        WY2t, WX2t = _sep_tables(m2, 50)           # [128, 50, 7] each
        lt3 = _build_dense_full(m3, 25, CH3)       # [128, 640, 49]

        cst = np.zeros((128, CST_COLS), np.float16)
        cst[:, :PAT0_COLS] = pat0.reshape(128, -1)

        idxs = np.zeros((128, IDX_COLS), np.int16)
        wd = np.zeros((NPAIR, 128, NTD, NBIN2), np.float32)

        # dense lhsT: L1 strips tiles 0..17 (vectorized scatter)
        _strip_scatter(wd, w1[sl], 0, NS1, 4, W1, 0, W1)
        _strip_scatter(wd, w1[sl], 1, NS1, 4, W1, 0, W1)
        # L3 tiles 18..22: dense full-map weights
        ltc = lt3.reshape(NPAIR, 2, CH3, 128, NBIN)
        wd[:, :, NT1:, :NBIN] = ltc[:, 0].transpose(0, 2, 1, 3)
        wd[:, :, NT1:, NBIN:] = ltc[:, 1].transpose(0, 2, 1, 3)
        # L2 separable tables per ROI: wy2 [128, 25, 7] (y = 2ch + p//64),
        # wx2 [128, 7] (x = p%64, zero for x >= 50)
        pidx = np.arange(128)
        ych = (2 * np.arange(CH2)[None, :] + (pidx[:, None] // 64))  # [128,25]
        wy2 = WY2t[:, ych, :]                       # [128rois,128p,25,7]
        xp = pidx % 64
        wx2 = np.where((xp < 50)[None, :, None],
                       WX2t[:, np.minimum(xp, 49), :], 0.0)  # [128rois,128p,7]
        wtab = np.concatenate(
            [wy2.reshape(NROI_CORE, 128, WY2), wx2], axis=2)  # [128,128,182]
        wtab = wtab.reshape(NPAIR, 2, 128, WY2 + POOLED)

        for p in range(NPAIR):
            ra, rb = s + 2 * p, s + 2 * p + 1
            # --- L0: scal table + idx
            j0 = np.full(N0, -1, np.int64)
            j0[:NS0] = idx0[ra]
            j0[NS0:2 * NS0] = idx0[rb]
            sc = np.zeros((CH0 * 128, W0), np.float32)
            wpair = np.concatenate([w0[ra], w0[rb]], axis=0)  # [392, 8, 7]
            bins_x = np.tile(np.arange(NS0) % POOLED, 2)
            sc[:2 * NS0] = wpair[np.arange(2 * NS0), :, bins_x]
            cst[:, PAT0_COLS + p * SCAL0_PER_PAIR:
                PAT0_COLS + (p + 1) * SCAL0_PER_PAIR] = (
                sc.reshape(CH0, 128, W0).transpose(1, 0, 2)
                .reshape(128, SCAL0_PER_PAIR).astype(np.float16))
            # --- idx
            j1 = np.empty(N1, np.int64)
            j1[:NS1] = idx1[ra]
            j1[NS1:] = idx1[rb]
            col = p * PAIR_ICOLS
            idxs[:, col:col + N0 // 16] = _pack_idx(j0)
            col += N0 // 16
            idxs[:, col:col + N1 // 16] = _pack_idx(j1)

        wrow = np.concatenate(
            [wd.reshape(NPAIR, 128, NTD * NBIN2),
             wtab[:, 0], wtab[:, 1]], axis=2)       # [NPAIR, 128, 2618]
        in_maps.append({
            "f0": feats[0][b], "f1": feats[1][b],
            "f2s": f2s[b], "f3s": f3s[b],
            "cst": cst, "idxs": idxs,
            "wd": wrow.astype(np.float16).reshape(
                NBLK, 2, 128, PAIR_WROW).transpose(0, 2, 1, 3).reshape(
                NBLK, 128, 2 * PAIR_WROW),
        })
    return in_maps


def _build_module():
    from concourse import bacc, tile
    from concourse.bass import mybir
    import concourse.bass as bass_mod

    F32 = mybir.dt.float32
    F16 = mybir.dt.float16
    I16 = mybir.dt.int16
    AP = bass_mod.AP

    nc = bacc.Bacc(None, target_bir_lowering=False, num_swdge_queues=4)
    f0 = nc.dram_tensor("f0", [F0_ROWS, C], F16, kind="ExternalInput")
    f1 = nc.dram_tensor("f1", [F1_ROWS, C], F16, kind="ExternalInput")
    f2s = nc.dram_tensor("f2s", [128, CH2 * C], F16, kind="ExternalInput")
    f3s = nc.dram_tensor("f3s", [128, CH3 * C], F16, kind="ExternalInput")
    cst = nc.dram_tensor("cst", [128, CST_COLS], F16, kind="ExternalInput")
    idxs = nc.dram_tensor("idxs", [128, IDX_COLS], I16, kind="ExternalInput")
    wd = nc.dram_tensor("wd", [NBLK, 128, 2 * PAIR_WROW], F16,
                        kind="ExternalInput")
    out = nc.dram_tensor("out", [NBIN2, NPAIR, C], F16, kind="ExternalOutput")

    # strided views for strip gathers (strides/sizes in fp16 elements)
    f0v = AP(f0, 0, [[2 * C, (F0_ROWS - W0) // 2 + 1], [1, W0 * C]])
    f1v = AP(f1, 0, [[C, F1_ROWS - W1 + 1], [1, W1 * C]])

    # split gather calls (src, elem, step, chunk lo/hi, idx lo/hi, nidx, q)
    CALLS = [
        (0, f0v, W0 * C, 2 * C, 0, 2, 0, 16, 256, 0),
        (0, f0v, W0 * C, 2 * C, 2, 4, 16, 25, 144, 2),
        (1, f1v, W1 * C, C, 0, 1, 25, 33, 128, 1),
        (1, f1v, W1 * C, C, 1, 2, 33, 39, 96, 3),
    ]

    with tile.TileContext(nc) as tc:
        with (
            tc.tile_pool(name="const", bufs=1) as constp,
            tc.tile_pool(name="g0p", bufs=3) as g0p,
            tc.tile_pool(name="g1p", bufs=3) as g1p,
            tc.tile_pool(name="w0p", bufs=4) as w0p,
            tc.tile_pool(name="wdp", bufs=3) as wdp,
            tc.tile_pool(name="accp", bufs=8, space="PSUM") as accp,
            tc.tile_pool(name="evp", bufs=3) as evp,
        ):
            cst_t = constp.tile([128, CST_COLS], F16)
            nc.sync.dma_start(cst_t[:], cst[:])
            idx_t = constp.tile([128, IDX_COLS], I16)
            nc.sync.dma_start(idx_t[:], idxs[:])
            f2t = constp.tile([128, CH2, C], F16)
            nc.sync.dma_start(f2t[:], f2s.rearrange("p (h c) -> p h c", h=CH2))
            f3t = constp.tile([128, CH3, C], F16)
            nc.sync.dma_start(f3t[:], f3s.rearrange("p (h c) -> p h c", h=CH3))

            gpools = [g0p, g1p]
            pat0_ap = cst_t[:, 0:PAT0_COLS]
            n_mm = CH0 * W0 + NT1 + CH2 + NT3

            for blk in range(NBLK):
                wdt = wdp.tile([128, 2, PAIR_WROW], F16, tag="wd")
                nc.sync.dma_start(wdt[:], wd[blk].rearrange(
                    "p (i t) -> p i t", i=2))
                ev = evp.tile([NBIN2, 2, C], F16, tag="ev")
                for half in range(2):
                    p = blk * 2 + half
                    col = p * PAIR_ICOLS
                    gt0 = gpools[0].tile([128, CH0, W0 * C], F16, tag="g0")
                    gt1 = gpools[1].tile([128, CH1, W1 * C], F16, tag="g1")
                    gts = [gt0, gt1]
                    if p < 3:
                        nc.vector.memset(gts[0][:], 0)
                        nc.vector.memset(gts[1][:], 0)
                    for l, src, elem, step, c0, c1, i0, i1, nidx, q in CALLS:
                        # alternate queues by pair parity to balance bytes
                        qn = q ^ (2 if (p & 1) else 0)
                        nc.gpsimd.dma_gather(
                            gts[l][:, c0:c1, :], src,
                            idx_t[:, col + i0:col + i1],
                            nidx, nidx, elem, elem_step=step, queue_num=qn)

                    # L0 weights: one broadcast DVE op
                    w0t = w0p.tile([128, CH0, W0, NBIN2], F16, tag="w0")
                    pat_b = (pat0_ap.rearrange("p (c b) -> p c b", c=CH0)
                             .unsqueeze(2).broadcast_to((128, CH0, W0, NBIN2)))
                    so = PAT0_COLS + p * SCAL0_PER_PAIR
                    scal_b = (cst_t[:, so:so + SCAL0_PER_PAIR]
                              .rearrange("p (c s) -> p c s", c=CH0)
                              .unsqueeze(3).broadcast_to((128, CH0, W0, NBIN2)))
                    nc.vector.tensor_tensor(w0t[:], pat_b, scal_b,
                                            mybir.AluOpType.mult)
                    # L2 weights: separable wy2 x wx2, one DVE op per half
                    w2t = w0p.tile([128, CH2, NBIN2], F16, tag="w2")
                    for h in range(2):
                        wyo = WYO + h * (WY2 + POOLED)
                        wy_b = (wdt[:, half, wyo:wyo + WY2]
                                .rearrange("p (c y) -> p c y", c=CH2)
                                .unsqueeze(3)
                                .broadcast_to((128, CH2, POOLED, POOLED)))
                        wx_b = (wdt[:, half, wyo + WY2:wyo + WY2 + POOLED]
                                .unsqueeze(1).unsqueeze(2)
                                .broadcast_to((128, CH2, POOLED, POOLED)))
                        nc.vector.tensor_tensor(
                            w2t[:, :, h * NBIN:(h + 1) * NBIN].rearrange(
                                "p c (a b) -> p c a b", a=POOLED),
                            wy_b, wx_b, mybir.AluOpType.mult)

                    acc = accp.tile([NBIN2, C], F32)
                    mi = 0
                    for c in range(CH0):
                        for sl in range(W0):
                            nc.tensor.matmul(
                                acc[:], w0t[:, c, sl, :],
                                gts[0][:, c, sl * C:(sl + 1) * C],
                                start=(mi == 0), stop=(mi == n_mm - 1))
                            mi += 1
                    for c in range(CH1):
                        for sl in range(W1):
                            t = c * W1 + sl
                            nc.tensor.matmul(
                                acc[:], wdt[:, half, t * NBIN2:(t + 1) * NBIN2],
                                gts[1][:, c, sl * C:(sl + 1) * C],
                                start=(mi == 0), stop=(mi == n_mm - 1))
                            mi += 1
                    for c in range(CH2):
                        nc.tensor.matmul(
                            acc[:], w2t[:, c, :], f2t[:, c, :],
                            start=(mi == 0), stop=(mi == n_mm - 1))
                        mi += 1
                    for c in range(CH3):
                        t = NT1 + c
                        nc.tensor.matmul(
                            acc[:], wdt[:, half, t * NBIN2:(t + 1) * NBIN2],
                            f3t[:, c, :],
                            start=(mi == 0), stop=(mi == n_mm - 1))
                        mi += 1

                    nc.scalar.copy(ev[:, half, :], acc[:])
                nc.sync.dma_start(out[:, 2 * blk:2 * blk + 2, :], ev[:])
    nc.finalize()
    return nc


def kernel(x0, x1, x2, x3, boxes):
    from concourse.bass_utils import run_bass_kernel_spmd
    in_maps = _host_prepare(x0, x1, x2, x3, boxes)
    if 'nc' not in _MODULE_CACHE:
        _MODULE_CACHE['nc'] = _build_module()
    nc = _MODULE_CACHE['nc']
    res = run_bass_kernel_spmd(nc, in_maps, list(range(8)))
    _MODULE_CACHE['last_result'] = res
    # per-core out is [98, 64, 256] bin-major: bin2 = half*49+bin
    parts = []
    for k in range(8):
        o = res.results[k]["out"].reshape(2, NBIN, NPAIR, C)
        parts.append(np.ascontiguousarray(
            o.transpose(2, 0, 3, 1)).reshape(NROI_CORE, C, NBIN))
    full = np.concatenate(parts, axis=0)           # [1024, 256, 49]
    return full.reshape(1024, C, POOLED, POOLED).astype(np.float32)


# revision 44
# speedup vs baseline: 1.0595x; 1.0595x over previous
"""Multi-level ROI Align (FPN pooler, 4 levels summed) on 8 Trainium2 cores.

v3.3: descriptor- and byte-minimized fp16 design. Shard ROIs across cores
(core k: batch k//4, 128 ROIs as 64 pairs). Host computes all gather
indices and bilinear weights from `boxes`; device does HBM strip-gathers
(dma_gather, one descriptor per multi-pixel fp16 strip, 4 SWDGE queues)
+ fp16 matmuls accumulating both ROIs of a pair into one PSUM tile
[98, 256] (bins 0-48 = ROI a, 49-97 = ROI b), evicted to DRAM bin-major
[98, 64pair, 256] fp32; host transposes.

Per pair-of-ROIs (62+ matmuls, ~616 gather descriptors):
  L0 (200x200, s=.25):  2x196 strips of 8px (even-aligned, idx=flat>>1),
      one per (ysample, rowsel, xbin); rank-1 weights (fixed one-hot bin
      pattern x per-strip scalar, one broadcast DVE op) -> 32 matmuls
  L1 (100x100, s=.125): 2x112 strips of 9px, one per (ysample, rowsel,
      xbin-pair); host-baked dense lhsT (DMA'd per block) -> 18 mm
  L2 (50x50, s=.0625):  NO gather -- full map SBUF-resident, padded to
      [50,64] rows (25 chunks); weights built on device from separable
      per-ROI wy[25ch,7]/wx[7] tables via 2 DVE ops -> 25 mm
  L3 (25x25, s=.03125): NO gather -- full map SBUF-resident (5 chunks);
      host-baked dense separable lhsT -> 5 mm
"""
import sys
import numpy as np

sys.path.insert(0, '/opt/trn_rl_repo')

POOLED = 7
SAMP = 2
NBIN = 49
C = 256
IMG = 800.0

NROI_CORE = 128
NPAIR = 64

# per-level geometry
L0 = dict(H=200, W=200, scale=0.25)
L1 = dict(H=100, W=100, scale=0.125)
L2 = dict(H=50, W=50, scale=0.0625)
L3 = dict(H=25, W=25, scale=0.03125)

W0, W1 = 8, 9                          # strip widths (px)
NS0, NS1 = 196, 112                    # strips per ROI
N0, N1 = 400, 224                      # padded pair nidx (%16, incl -1 tail)
CH0, CH1 = 4, 2                        # chunks per pair
CH2 = 25                               # L2 chunks: [50, 64]-padded map
CH3 = 5                                # L3 full-map chunks (625px -> 5x128)
NT1, NT3 = CH1 * W1, CH3               # dense lhsT tiles (L1, L3)
NTD = NT1 + NT3                        # 18 + 5 = 23
NBIN2 = 2 * NBIN                       # 98
WY2 = CH2 * POOLED                     # 175 wy2 cols per ROI
PAIR_WROW = NTD * NBIN2 + 2 * (WY2 + POOLED)   # 2618 wd els per pair
WYO = NTD * NBIN2                      # wy2/wx2 table offset in wd row
NBLK = NPAIR // 2                      # 2-pair blocks

F0_ROWS = 40004
F1_ROWS = 10000
F2_ROWS = 2560
F3_ROWS = 640

# cst fp16 column layout: L0 pattern [4ch, 98] then L0 scalars
PAT0_COLS = CH0 * NBIN2                        # 392
SCAL0_PER_PAIR = CH0 * W0                      # 32
CST_COLS = PAT0_COLS + NPAIR * SCAL0_PER_PAIR  # 392 + 2048

# idx int16 layout per pair: [L0 400][L1 224] / 16
PAIR_ICOLS = (N0 + N1) // 16                   # 39
IDX_COLS = NPAIR * PAIR_ICOLS

_MODULE_CACHE = {}


def _sample_meta(boxes_b, H, W, scale):
    """Per-ROI sample geometry in fp32, matching reference op order.
    boxes_b: [N, 4] fp32. Returns dict of [N,7,2] arrays."""
    f = np.float32
    b = boxes_b.astype(np.float32)
    x1 = b[:, 0] * f(scale)
    y1 = b[:, 1] * f(scale)
    x2 = b[:, 2] * f(scale)
    y2 = b[:, 3] * f(scale)
    rw = np.maximum(x2 - x1, f(1.0))
    rh = np.maximum(y2 - y1, f(1.0))
    bw = rw / f(POOLED)
    bh = rh / f(POOLED)
    g = (np.arange(POOLED, dtype=np.float32)[:, None]
         + (np.arange(SAMP, dtype=np.float32)[None, :] + f(0.5)) / f(SAMP))
    y = y1[:, None, None] + g[None] * bh[:, None, None]   # [N,7,2]
    x = x1[:, None, None] + g[None] * bw[:, None, None]
    masky = ((y >= f(-1.0)) & (y <= f(H))).astype(np.float32)
    maskx = ((x >= f(-1.0)) & (x <= f(W))).astype(np.float32)
    yc = np.clip(y, f(0.0), f(H - 1))
    xc = np.clip(x, f(0.0), f(W - 1))
    yl = np.floor(yc).astype(np.int64)
    xl = np.floor(xc).astype(np.int64)
    yh = np.minimum(yl + 1, H - 1)
    xh = np.minimum(xl + 1, W - 1)
    ly = (yc - yl.astype(np.float32)).astype(np.float32)
    lx = (xc - xl.astype(np.float32)).astype(np.float32)
    hy = (f(1.0) - ly).astype(np.float32)
    hx = (f(1.0) - lx).astype(np.float32)
    return dict(yl=yl, yh=yh, xl=xl, xh=xh, ly=ly, lx=lx, hy=hy, hx=hx,
                masky=masky, maskx=maskx, x=x, y=y)


def _strip_grid(meta, W, bins_per_strip, width, parity):
    """Build per-ROI strip indices and slot weights.

    Strips: (rowsel 2) x (ysample 14) x (xgroup ceil(7/b)).
    Returns idx [N, NS] int64 (pixel index of strip start, or start>>1 if
    parity), wslot [N, NS, width] fp32 (bilinear x-weights x y-weight x
    mask x 0.25), and bins [NS_xgroups arrays] for pattern building is
    implicit: each strip's samples' bins vary within the group -- handled
    by caller via per-sample info: also returns sample slot/bin arrays:
    contrib = (slotpos [N, NS, 7bins?..]) -- instead we return dense
    per-strip per-slot per-bin weights only when needed. For rank-1 (b=1)
    wslot is enough (all slots -> the strip's single bin).
    """
    N = meta['yl'].shape[0]
    f = np.float32
    ngrp = -(-POOLED // bins_per_strip)           # x-groups per row
    NS = 2 * 14 * ngrp
    # rows/yweights: [N, 2, 14]
    rows = np.stack([meta['yl'], meta['yh']], axis=1).reshape(N, 2, 14)
    wy = (np.stack([meta['hy'], meta['ly']], axis=1)
          * meta['masky'][:, None]).reshape(N, 2, 14).astype(np.float32)
    # x corners per sample: [N, 7, 2]
    xl, xh = meta['xl'], meta['xh']
    wxl = (meta['hx'] * meta['maskx']).astype(np.float32)
    wxh = (meta['lx'] * meta['maskx']).astype(np.float32)
    # group starts: min xl over samples in group -> [N, ngrp]
    xs = np.empty((N, ngrp), np.int64)
    for gi in range(ngrp):
        b0, b1 = gi * bins_per_strip, min((gi + 1) * bins_per_strip, POOLED)
        xs[:, gi] = xl[:, b0:b1, :].reshape(N, -1).min(axis=1)
    if parity:
        xs &= ~1
    xs = np.clip(xs, 0, W - width)
    # slot weights [N, ngrp, width] per (bin-in-group, sx): scatter
    wslot = np.zeros((N, ngrp, width, POOLED), np.float32)  # per-bin slots
    ridx = np.arange(N)[:, None, None]
    for gi in range(ngrp):
        b0, b1 = gi * bins_per_strip, min((gi + 1) * bins_per_strip, POOLED)
        for bx in range(b0, b1):
            for sx in range(SAMP):
                ol = xl[:, bx, sx] - xs[:, gi]
                oh = xh[:, bx, sx] - xs[:, gi]
                np.add.at(wslot, (np.arange(N), gi, ol, bx), wxl[:, bx, sx])
                np.add.at(wslot, (np.arange(N), gi, oh, bx), wxh[:, bx, sx])
    # combine with y: strips ordered (ysample, rowsel, xgroup) so the
    # yl/yh descriptors of one sample hit adjacent DRAM rows back-to-back
    idx = (rows[:, :, :, None] * W + xs[:, None, None, :])
    if parity:
        idx >>= 1
    idx = idx.transpose(0, 2, 1, 3).reshape(N, NS)       # [N,14,2,ngrp]
    w = (wy[:, :, :, None, None, None]
         * wslot[:, None, None, :, :, :] * f(0.25))      # [N,2,14,g,w,7]
    w = w.transpose(0, 2, 1, 3, 4, 5).reshape(N, NS, width, POOLED)
    return idx, w


def _sep_tables(meta, HW):
    """Separable bilinear weight tables WY/WX [N, HW, 7] fp32."""
    N = meta['yl'].shape[0]
    f = np.float32
    WY = np.zeros((N, HW, POOLED), np.float32)
    WX = np.zeros((N, HW, POOLED), np.float32)
    ridx = np.arange(N)[:, None, None]
    pidx = np.broadcast_to(np.arange(POOLED)[None, :, None], (N, POOLED, SAMP))
    np.add.at(WY, (ridx, meta['yl'], pidx),
              (f(0.5) * meta['hy'] * meta['masky']).astype(np.float32))
    np.add.at(WY, (ridx, meta['yh'], pidx),
              (f(0.5) * meta['ly'] * meta['masky']).astype(np.float32))
    np.add.at(WX, (ridx, meta['xl'], pidx),
              (f(0.5) * meta['hx'] * meta['maskx']).astype(np.float32))
    np.add.at(WX, (ridx, meta['xh'], pidx),
              (f(0.5) * meta['lx'] * meta['maskx']).astype(np.float32))
    return WY, WX


def _build_dense_full(meta, HW, nch):
    """Dense separable weights over the FULL HWxHW map (features live in
    SBUF on device). Returns lhsT [N, nch*128, 49] fp32 (px zero-padded)."""
    N = meta['yl'].shape[0]
    WY, WX = _sep_tables(meta, HW)
    lhsT = np.einsum('nap,nbq->nabpq', WY, WX).reshape(N, HW * HW, NBIN)
    out = np.zeros((N, nch * 128, NBIN), np.float32)
    out[:, :HW * HW] = lhsT
    return out


def _pack_idx(idx_flat):
    """[n] int -> [128, n//16] int16: wrap 16 partitions, replicate 8x."""
    n = idx_flat.shape[0]
    arr = idx_flat.reshape(n // 16, 16).T            # [16, cols]
    arr = np.broadcast_to(arr[None], (8, 16, n // 16)).reshape(128, n // 16)
    return arr.astype(np.int16)


def _l0_pattern():
    """Fixed one-hot [128, CH0, 98] fp16: J = c*128+p -> bin.
    j order within an ROI: (ysample, rowsel, xbin)."""
    pat = np.zeros((CH0, 128, NBIN2), np.float16)
    for J in range(2 * NS0):
        half, j = J // NS0, J % NS0
        bx = j % POOLED
        t = j // (2 * POOLED)
        py = t // 2
        pat[J // 128, J % 128, py * 7 + bx + half * NBIN] = 1.0
    return pat.transpose(1, 0, 2)                    # [128, CH0, 98]


def _strip_scatter(wd, w, half, ns, ngrp, width, t_base, chunks_w):
    """Scatter per-ROI strip weights [NROI_CORE, ns, width, 7] into wd
    [NPAIR, 128, NTD, 98]. half: 0/1 (roi parity within pair)."""
    jj = np.arange(ns)
    J = half * ns + jj
    ch, pp = J // 128, J % 128
    py = (jj // (2 * ngrp)) // 2
    t = t_base + ch[:, None] * chunks_w + np.arange(width)[None, :]  # [ns, width]
    bn = py[:, None] * 7 + np.arange(POOLED)[None, :] + half * NBIN  # [ns, 7]
    rois = np.arange(half, NROI_CORE, 2)
    pair_i = np.broadcast_to((rois // 2)[:, None, None, None],
                             (NPAIR, ns, width, POOLED))
    pp_i = np.broadcast_to(pp[None, :, None, None], pair_i.shape)
    t_i = np.broadcast_to(t[None, :, :, None], pair_i.shape)
    bn_i = np.broadcast_to(bn[None, :, None, :], pair_i.shape)
    np.add.at(wd, (pair_i, pp_i, t_i, bn_i), w[rois])


def _host_prepare(x0, x1, x2, x3, boxes):
    """Build all per-core input tensors. Returns list of 8 dicts."""
    B = boxes.shape[0]
    feats = []
    for arr, lv, rows in ((x0, L0, F0_ROWS), (x1, L1, F1_ROWS),
                          (x2, L2, F2_ROWS), (x3, L3, F3_ROWS)):
        f = np.zeros((B, rows, C), np.float16)
        hw = lv['H'] * lv['W']
        f[:, :hw] = np.ascontiguousarray(
            np.transpose(np.asarray(arr, np.float32), (0, 2, 3, 1))
        ).reshape(B, hw, C).astype(np.float16)
        feats.append(f)
    # f2 padded to [50, 64] rows so px = y*64+x -> y = 2ch + p//64, x = p%64
    f2pad = np.zeros((B, 50, 64, C), np.float16)
    f2pad[:, :, :50] = feats[2][:, :2500].reshape(B, 50, 50, C)
    f2s = np.ascontiguousarray(
        f2pad.reshape(B, CH2, 128, C).transpose(0, 2, 1, 3))
    # f3 for SBUF residency: [128, CH3, C], px = ch*128 + p
    f3s = np.ascontiguousarray(
        feats[3].reshape(B, CH3, 128, C).transpose(0, 2, 1, 3))

    pat0 = _l0_pattern()

    per_batch = []
    for b in range(B):
        bb = np.asarray(boxes[b], np.float32)
        m0 = _sample_meta(bb, L0['H'], L0['W'], L0['scale'])
        m1 = _sample_meta(bb, L1['H'], L1['W'], L1['scale'])
        idx0, w0 = _strip_grid(m0, L0['W'], 1, W0, parity=True)   # [N,196],[N,196,8,7]
        idx1, w1 = _strip_grid(m1, L1['W'], 2, W1, parity=False)  # [N,112],[N,112,9,7]
        per_batch.append((idx0, w0, idx1, w1))

    in_maps = []
    for k in range(8):
        b = k // 4
        s = (k % 4) * NROI_CORE
        idx0, w0, idx1, w1 = per_batch[b]
        sl = slice(s, s + NROI_CORE)
        bb = np.asarray(boxes[b][sl], np.float32)
        m2 = _sample_meta(bb, L2['H'], L2['W'], L2['scale'])
        m3 = _sample_meta(bb, L3['H'], L3['W'], L3['scale'])
        WY2t, WX2t = _sep_tables(m2, 50)           # [128, 50, 7] each
        lt3 = _build_dense_full(m3, 25, CH3)       # [128, 640, 49]

        cst = np.zeros((128, CST_COLS), np.float16)
        cst[:, :PAT0_COLS] = pat0.reshape(128, -1)

        idxs = np.zeros((128, IDX_COLS), np.int16)
        wd = np.zeros((NPAIR, 128, NTD, NBIN2), np.float32)

        # dense lhsT: L1 strips tiles 0..17 (vectorized scatter)
        _strip_scatter(wd, w1[sl], 0, NS1, 4, W1, 0, W1)
        _strip_scatter(wd, w1[sl], 1, NS1, 4, W1, 0, W1)
        # L3 tiles 18..22: dense full-map weights
        ltc = lt3.reshape(NPAIR, 2, CH3, 128, NBIN)
        wd[:, :, NT1:, :NBIN] = ltc[:, 0].transpose(0, 2, 1, 3)
        wd[:, :, NT1:, NBIN:] = ltc[:, 1].transpose(0, 2, 1, 3)
        # L2 separable tables per ROI: wy2 [128, 25, 7] (y = 2ch + p//64),
        # wx2 [128, 7] (x = p%64, zero for x >= 50)
        pidx = np.arange(128)
        ych = (2 * np.arange(CH2)[None, :] + (pidx[:, None] // 64))  # [128,25]
        wy2 = WY2t[:, ych, :]                       # [128rois,128p,25,7]
        xp = pidx % 64
        wx2 = np.where((xp < 50)[None, :, None],
                       WX2t[:, np.minimum(xp, 49), :], 0.0)  # [128rois,128p,7]
        wtab = np.concatenate(
            [wy2.reshape(NROI_CORE, 128, WY2), wx2], axis=2)  # [128,128,182]
        wtab = wtab.reshape(NPAIR, 2, 128, WY2 + POOLED)

        for p in range(NPAIR):
            ra, rb = s + 2 * p, s + 2 * p + 1
            # --- L0: scal table + idx
            j0 = np.full(N0, -1, np.int64)
            j0[:NS0] = idx0[ra]
            j0[NS0:2 * NS0] = idx0[rb]
            sc = np.zeros((CH0 * 128, W0), np.float32)
            wpair = np.concatenate([w0[ra], w0[rb]], axis=0)  # [392, 8, 7]
            bins_x = np.tile(np.arange(NS0) % POOLED, 2)
            sc[:2 * NS0] = wpair[np.arange(2 * NS0), :, bins_x]
            cst[:, PAT0_COLS + p * SCAL0_PER_PAIR:
                PAT0_COLS + (p + 1) * SCAL0_PER_PAIR] = (
                sc.reshape(CH0, 128, W0).transpose(1, 0, 2)
                .reshape(128, SCAL0_PER_PAIR).astype(np.float16))
            # --- idx
            j1 = np.empty(N1, np.int64)
            j1[:NS1] = idx1[ra]
            j1[NS1:] = idx1[rb]
            col = p * PAIR_ICOLS
            idxs[:, col:col + N0 // 16] = _pack_idx(j0)
            col += N0 // 16
            idxs[:, col:col + N1 // 16] = _pack_idx(j1)

        wrow = np.concatenate(
            [wd.reshape(NPAIR, 128, NTD * NBIN2),
             wtab[:, 0], wtab[:, 1]], axis=2)       # [NPAIR, 128, 2618]
        in_maps.append({
            "f0": feats[0][b], "f1": feats[1][b],
            "f2s": f2s[b], "f3s": f3s[b],
            "cst": cst, "idxs": idxs,
            "wd": wrow.astype(np.float16).reshape(
                NBLK, 2, 128, PAIR_WROW).transpose(0, 2, 1, 3).reshape(
                NBLK, 128, 2 * PAIR_WROW),
        })
    return in_maps


def _build_module():
    from concourse import bacc, tile
    from concourse.bass import mybir
    import concourse.bass as bass_mod

    F32 = mybir.dt.float32
    F16 = mybir.dt.float16
    I16 = mybir.dt.int16
    AP = bass_mod.AP

    nc = bacc.Bacc(None, target_bir_lowering=False, num_swdge_queues=4)
    f0 = nc.dram_tensor("f0", [F0_ROWS, C], F16, kind="ExternalInput")
    f1 = nc.dram_tensor("f1", [F1_ROWS, C], F16, kind="ExternalInput")
    f2s = nc.dram_tensor("f2s", [128, CH2 * C], F16, kind="ExternalInput")
    f3s = nc.dram_tensor("f3s", [128, CH3 * C], F16, kind="ExternalInput")
    cst = nc.dram_tensor("cst", [128, CST_COLS], F16, kind="ExternalInput")
    idxs = nc.dram_tensor("idxs", [128, IDX_COLS], I16, kind="ExternalInput")
    wd = nc.dram_tensor("wd", [NBLK, 128, 2 * PAIR_WROW], F16,
                        kind="ExternalInput")
    out = nc.dram_tensor("out", [NBIN2, NPAIR, C], F16, kind="ExternalOutput")

    # strided views for strip gathers (strides/sizes in fp16 elements)
    f0v = AP(f0, 0, [[2 * C, (F0_ROWS - W0) // 2 + 1], [1, W0 * C]])
    f1v = AP(f1, 0, [[C, F1_ROWS - W1 + 1], [1, W1 * C]])

    # split gather calls (src, elem, step, chunk lo/hi, idx lo/hi, nidx, q)
    CALLS = [
        (0, f0v, W0 * C, 2 * C, 0, 2, 0, 16, 256, 0),
        (0, f0v, W0 * C, 2 * C, 2, 4, 16, 25, 144, 2),
        (1, f1v, W1 * C, C, 0, 1, 25, 33, 128, 1),
        (1, f1v, W1 * C, C, 1, 2, 33, 39, 96, 3),
    ]

    with tile.TileContext(nc) as tc:
        with (
            tc.tile_pool(name="const", bufs=1) as constp,
            tc.tile_pool(name="g0p", bufs=3) as g0p,
            tc.tile_pool(name="g1p", bufs=3) as g1p,
            tc.tile_pool(name="w0p", bufs=4) as w0p,
            tc.tile_pool(name="wdp", bufs=3) as wdp,
            tc.tile_pool(name="accp", bufs=8, space="PSUM") as accp,
            tc.tile_pool(name="evp", bufs=3) as evp,
        ):
            idx_t = constp.tile([128, IDX_COLS], I16)
            nc.sync.dma_start(idx_t[:], idxs[:])
            cst_t = constp.tile([128, CST_COLS], F16)
            nc.sync.dma_start(cst_t[:], cst[:])
            f2t = constp.tile([128, CH2, C], F16)
            nc.sync.dma_start(f2t[:], f2s.rearrange("p (h c) -> p h c", h=CH2))
            f3t = constp.tile([128, CH3, C], F16)
            nc.sync.dma_start(f3t[:], f3s.rearrange("p (h c) -> p h c", h=CH3))

            gpools = [g0p, g1p]
            pat0_ap = cst_t[:, 0:PAT0_COLS]
            n_mm = CH0 * W0 + NT1 + CH2 + NT3

            for blk in range(NBLK):
                wdt = wdp.tile([128, 2, PAIR_WROW], F16, tag="wd")
                nc.sync.dma_start(wdt[:], wd[blk].rearrange(
                    "p (i t) -> p i t", i=2))
                ev = evp.tile([NBIN2, 2, C], F16, tag="ev")
                for half in range(2):
                    p = blk * 2 + half
                    col = p * PAIR_ICOLS
                    gt0 = gpools[0].tile([128, CH0, W0 * C], F16, tag="g0")
                    gt1 = gpools[1].tile([128, CH1, W1 * C], F16, tag="g1")
                    gts = [gt0, gt1]
                    if p < 3:
                        nc.vector.memset(gts[0][:], 0)
                        nc.vector.memset(gts[1][:], 0)
                    for l, src, elem, step, c0, c1, i0, i1, nidx, q in CALLS:
                        # alternate queues by pair parity to balance bytes
                        qn = q ^ (2 if (p & 1) else 0)
                        nc.gpsimd.dma_gather(
                            gts[l][:, c0:c1, :], src,
                            idx_t[:, col + i0:col + i1],
                            nidx, nidx, elem, elem_step=step, queue_num=qn,
                            single_packet=False)

                    # L0 weights: one broadcast DVE op
                    w0t = w0p.tile([128, CH0, W0, NBIN2], F16, tag="w0")
                    pat_b = (pat0_ap.rearrange("p (c b) -> p c b", c=CH0)
                             .unsqueeze(2).broadcast_to((128, CH0, W0, NBIN2)))
                    so = PAT0_COLS + p * SCAL0_PER_PAIR
                    scal_b = (cst_t[:, so:so + SCAL0_PER_PAIR]
                              .rearrange("p (c s) -> p c s", c=CH0)
                              .unsqueeze(3).broadcast_to((128, CH0, W0, NBIN2)))
                    nc.vector.tensor_tensor(w0t[:], pat_b, scal_b,
                                            mybir.AluOpType.mult)
                    # L2 weights: separable wy2 x wx2, one DVE op per half
                    w2t = w0p.tile([128, CH2, NBIN2], F16, tag="w2")
                    for h in range(2):
                        wyo = WYO + h * (WY2 + POOLED)
                        wy_b = (wdt[:, half, wyo:wyo + WY2]
                                .rearrange("p (c y) -> p c y", c=CH2)
                                .unsqueeze(3)
                                .broadcast_to((128, CH2, POOLED, POOLED)))
                        wx_b = (wdt[:, half, wyo + WY2:wyo + WY2 + POOLED]
                                .unsqueeze(1).unsqueeze(2)
                                .broadcast_to((128, CH2, POOLED, POOLED)))
                        nc.vector.tensor_tensor(
                            w2t[:, :, h * NBIN:(h + 1) * NBIN].rearrange(
                                "p c (a b) -> p c a b", a=POOLED),
                            wy_b, wx_b, mybir.AluOpType.mult)

                    acc = accp.tile([NBIN2, C], F32)
                    mi = 0
                    for c in range(CH0):
                        for sl in range(W0):
                            nc.tensor.matmul(
                                acc[:], w0t[:, c, sl, :],
                                gts[0][:, c, sl * C:(sl + 1) * C],
                                start=(mi == 0), stop=(mi == n_mm - 1))
                            mi += 1
                    for c in range(CH1):
                        for sl in range(W1):
                            t = c * W1 + sl
                            nc.tensor.matmul(
                                acc[:], wdt[:, half, t * NBIN2:(t + 1) * NBIN2],
                                gts[1][:, c, sl * C:(sl + 1) * C],
                                start=(mi == 0), stop=(mi == n_mm - 1))
                            mi += 1
                    for c in range(CH2):
                        nc.tensor.matmul(
                            acc[:], w2t[:, c, :], f2t[:, c, :],
                            start=(mi == 0), stop=(mi == n_mm - 1))
                        mi += 1
                    for c in range(CH3):
                        t = NT1 + c
                        nc.tensor.matmul(
                            acc[:], wdt[:, half, t * NBIN2:(t + 1) * NBIN2],
                            f3t[:, c, :],
                            start=(mi == 0), stop=(mi == n_mm - 1))
                        mi += 1

                    nc.scalar.copy(ev[:, half, :], acc[:])
                nc.sync.dma_start(out[:, 2 * blk:2 * blk + 2, :], ev[:])
    nc.finalize()
    return nc


def kernel(x0, x1, x2, x3, boxes):
    from concourse.bass_utils import run_bass_kernel_spmd
    in_maps = _host_prepare(x0, x1, x2, x3, boxes)
    if 'nc' not in _MODULE_CACHE:
        _MODULE_CACHE['nc'] = _build_module()
    nc = _MODULE_CACHE['nc']
    res = run_bass_kernel_spmd(nc, in_maps, list(range(8)))
    _MODULE_CACHE['last_result'] = res
    # per-core out is [98, 64, 256] bin-major: bin2 = half*49+bin
    parts = []
    for k in range(8):
        o = res.results[k]["out"].reshape(2, NBIN, NPAIR, C)
        parts.append(np.ascontiguousarray(
            o.transpose(2, 0, 3, 1)).reshape(NROI_CORE, C, NBIN))
    full = np.concatenate(parts, axis=0)           # [1024, 256, 49]
    return full.reshape(1024, C, POOLED, POOLED).astype(np.float32)


# revision 46
# speedup vs baseline: 1.1107x; 1.0483x over previous
"""Multi-level ROI Align (FPN pooler, 4 levels summed) on 8 Trainium2 cores.

v3.3: descriptor- and byte-minimized fp16 design. Shard ROIs across cores
(core k: batch k//4, 128 ROIs as 64 pairs). Host computes all gather
indices and bilinear weights from `boxes`; device does HBM strip-gathers
(dma_gather, one descriptor per multi-pixel fp16 strip, 4 SWDGE queues)
+ fp16 matmuls accumulating both ROIs of a pair into one PSUM tile
[98, 256] (bins 0-48 = ROI a, 49-97 = ROI b), evicted to DRAM bin-major
[98, 64pair, 256] fp32; host transposes.

Per pair-of-ROIs (62+ matmuls, ~616 gather descriptors):
  L0 (200x200, s=.25):  2x196 strips of 8px (even-aligned, idx=flat>>1),
      one per (ysample, rowsel, xbin); rank-1 weights (fixed one-hot bin
      pattern x per-strip scalar, one broadcast DVE op) -> 32 matmuls
  L1 (100x100, s=.125): 2x112 strips of 9px, one per (ysample, rowsel,
      xbin-pair); host-baked dense lhsT (DMA'd per block) -> 18 mm
  L2 (50x50, s=.0625):  NO gather -- full map SBUF-resident, padded to
      [50,64] rows (25 chunks); weights built on device from separable
      per-ROI wy[25ch,7]/wx[7] tables via 2 DVE ops -> 25 mm
  L3 (25x25, s=.03125): NO gather -- full map SBUF-resident (5 chunks);
      host-baked dense separable lhsT -> 5 mm
"""
import sys
import numpy as np

sys.path.insert(0, '/opt/trn_rl_repo')

POOLED = 7
SAMP = 2
NBIN = 49
C = 256
IMG = 800.0

NROI_CORE = 128
NPAIR = 64

# per-level geometry
L0 = dict(H=200, W=200, scale=0.25)
L1 = dict(H=100, W=100, scale=0.125)
L2 = dict(H=50, W=50, scale=0.0625)
L3 = dict(H=25, W=25, scale=0.03125)

W0, W1 = 8, 9                          # strip widths (px)
NS0, NS1 = 196, 112                    # strips per ROI
N0, N1 = 400, 224                      # padded pair nidx (%16, incl -1 tail)
CH0, CH1 = 4, 2                        # chunks per pair
CH2 = 25                               # L2 chunks: [50, 64]-padded map
CH3 = 7                                # L3 chunks: [25, 32]-padded map
NT1 = CH1 * W1                         # dense lhsT tiles (L1)
NTD = NT1                              # 18
NBIN2 = 2 * NBIN                       # 98
WY2 = CH2 * POOLED                     # 175 wy2 cols per ROI
WY3 = CH3 * POOLED                     # 49 wy3 cols per ROI
TABH = WY2 + POOLED + WY3 + POOLED     # 238 table els per ROI half
PAIR_WROW = NTD * NBIN2 + 2 * TABH     # 2240 wd els per pair
WYO = NTD * NBIN2                      # table offset in wd row
NBLK = NPAIR // 2                      # 2-pair blocks

F0_ROWS = 40004
F1_ROWS = 10000
F2_ROWS = 2560
F3_ROWS = 896

# cst fp16 column layout: L0 pattern [4ch, 98] then L0 scalars
PAT0_COLS = CH0 * NBIN2                        # 392
SCAL0_PER_PAIR = CH0 * W0                      # 32
CST_COLS = PAT0_COLS + NPAIR * SCAL0_PER_PAIR  # 392 + 2048

# idx int16 layout per pair: [L0 400][L1 224] / 16
PAIR_ICOLS = (N0 + N1) // 16                   # 39
IDX_COLS = NPAIR * PAIR_ICOLS

_MODULE_CACHE = {}


def _sample_meta(boxes_b, H, W, scale):
    """Per-ROI sample geometry in fp32, matching reference op order.
    boxes_b: [N, 4] fp32. Returns dict of [N,7,2] arrays."""
    f = np.float32
    b = boxes_b.astype(np.float32)
    x1 = b[:, 0] * f(scale)
    y1 = b[:, 1] * f(scale)
    x2 = b[:, 2] * f(scale)
    y2 = b[:, 3] * f(scale)
    rw = np.maximum(x2 - x1, f(1.0))
    rh = np.maximum(y2 - y1, f(1.0))
    bw = rw / f(POOLED)
    bh = rh / f(POOLED)
    g = (np.arange(POOLED, dtype=np.float32)[:, None]
         + (np.arange(SAMP, dtype=np.float32)[None, :] + f(0.5)) / f(SAMP))
    y = y1[:, None, None] + g[None] * bh[:, None, None]   # [N,7,2]
    x = x1[:, None, None] + g[None] * bw[:, None, None]
    masky = ((y >= f(-1.0)) & (y <= f(H))).astype(np.float32)
    maskx = ((x >= f(-1.0)) & (x <= f(W))).astype(np.float32)
    yc = np.clip(y, f(0.0), f(H - 1))
    xc = np.clip(x, f(0.0), f(W - 1))
    yl = np.floor(yc).astype(np.int64)
    xl = np.floor(xc).astype(np.int64)
    yh = np.minimum(yl + 1, H - 1)
    xh = np.minimum(xl + 1, W - 1)
    ly = (yc - yl.astype(np.float32)).astype(np.float32)
    lx = (xc - xl.astype(np.float32)).astype(np.float32)
    hy = (f(1.0) - ly).astype(np.float32)
    hx = (f(1.0) - lx).astype(np.float32)
    return dict(yl=yl, yh=yh, xl=xl, xh=xh, ly=ly, lx=lx, hy=hy, hx=hx,
                masky=masky, maskx=maskx, x=x, y=y)


def _strip_grid(meta, W, bins_per_strip, width, parity):
    """Build per-ROI strip indices and slot weights.

    Strips: (rowsel 2) x (ysample 14) x (xgroup ceil(7/b)).
    Returns idx [N, NS] int64 (pixel index of strip start, or start>>1 if
    parity), wslot [N, NS, width] fp32 (bilinear x-weights x y-weight x
    mask x 0.25), and bins [NS_xgroups arrays] for pattern building is
    implicit: each strip's samples' bins vary within the group -- handled
    by caller via per-sample info: also returns sample slot/bin arrays:
    contrib = (slotpos [N, NS, 7bins?..]) -- instead we return dense
    per-strip per-slot per-bin weights only when needed. For rank-1 (b=1)
    wslot is enough (all slots -> the strip's single bin).
    """
    N = meta['yl'].shape[0]
    f = np.float32
    ngrp = -(-POOLED // bins_per_strip)           # x-groups per row
    NS = 2 * 14 * ngrp
    # rows/yweights: [N, 2, 14]
    rows = np.stack([meta['yl'], meta['yh']], axis=1).reshape(N, 2, 14)
    wy = (np.stack([meta['hy'], meta['ly']], axis=1)
          * meta['masky'][:, None]).reshape(N, 2, 14).astype(np.float32)
    # x corners per sample: [N, 7, 2]
    xl, xh = meta['xl'], meta['xh']
    wxl = (meta['hx'] * meta['maskx']).astype(np.float32)
    wxh = (meta['lx'] * meta['maskx']).astype(np.float32)
    # group starts: min xl over samples in group -> [N, ngrp]
    xs = np.empty((N, ngrp), np.int64)
    for gi in range(ngrp):
        b0, b1 = gi * bins_per_strip, min((gi + 1) * bins_per_strip, POOLED)
        xs[:, gi] = xl[:, b0:b1, :].reshape(N, -1).min(axis=1)
    if parity:
        xs &= ~1
    xs = np.clip(xs, 0, W - width)
    # slot weights [N, ngrp, width] per (bin-in-group, sx): scatter
    wslot = np.zeros((N, ngrp, width, POOLED), np.float32)  # per-bin slots
    ridx = np.arange(N)[:, None, None]
    for gi in range(ngrp):
        b0, b1 = gi * bins_per_strip, min((gi + 1) * bins_per_strip, POOLED)
        for bx in range(b0, b1):
            for sx in range(SAMP):
                ol = xl[:, bx, sx] - xs[:, gi]
                oh = xh[:, bx, sx] - xs[:, gi]
                np.add.at(wslot, (np.arange(N), gi, ol, bx), wxl[:, bx, sx])
                np.add.at(wslot, (np.arange(N), gi, oh, bx), wxh[:, bx, sx])
    # combine with y: strips ordered (ysample, rowsel, xgroup) so the
    # yl/yh descriptors of one sample hit adjacent DRAM rows back-to-back
    idx = (rows[:, :, :, None] * W + xs[:, None, None, :])
    if parity:
        idx >>= 1
    idx = idx.transpose(0, 2, 1, 3).reshape(N, NS)       # [N,14,2,ngrp]
    w = (wy[:, :, :, None, None, None]
         * wslot[:, None, None, :, :, :] * f(0.25))      # [N,2,14,g,w,7]
    w = w.transpose(0, 2, 1, 3, 4, 5).reshape(N, NS, width, POOLED)
    return idx, w


def _sep_tables(meta, HW):
    """Separable bilinear weight tables WY/WX [N, HW, 7] fp32."""
    N = meta['yl'].shape[0]
    f = np.float32
    WY = np.zeros((N, HW, POOLED), np.float32)
    WX = np.zeros((N, HW, POOLED), np.float32)
    ridx = np.arange(N)[:, None, None]
    pidx = np.broadcast_to(np.arange(POOLED)[None, :, None], (N, POOLED, SAMP))
    np.add.at(WY, (ridx, meta['yl'], pidx),
              (f(0.5) * meta['hy'] * meta['masky']).astype(np.float32))
    np.add.at(WY, (ridx, meta['yh'], pidx),
              (f(0.5) * meta['ly'] * meta['masky']).astype(np.float32))
    np.add.at(WX, (ridx, meta['xl'], pidx),
              (f(0.5) * meta['hx'] * meta['maskx']).astype(np.float32))
    np.add.at(WX, (ridx, meta['xh'], pidx),
              (f(0.5) * meta['lx'] * meta['maskx']).astype(np.float32))
    return WY, WX


def _build_dense_full(meta, HW, nch):
    """Dense separable weights over the FULL HWxHW map (features live in
    SBUF on device). Returns lhsT [N, nch*128, 49] fp32 (px zero-padded)."""
    N = meta['yl'].shape[0]
    WY, WX = _sep_tables(meta, HW)
    lhsT = np.einsum('nap,nbq->nabpq', WY, WX).reshape(N, HW * HW, NBIN)
    out = np.zeros((N, nch * 128, NBIN), np.float32)
    out[:, :HW * HW] = lhsT
    return out


def _pack_idx(idx_flat):
    """[n] int -> [128, n//16] int16: wrap 16 partitions, replicate 8x."""
    n = idx_flat.shape[0]
    arr = idx_flat.reshape(n // 16, 16).T            # [16, cols]
    arr = np.broadcast_to(arr[None], (8, 16, n // 16)).reshape(128, n // 16)
    return arr.astype(np.int16)


def _l0_pattern():
    """Fixed one-hot [128, CH0, 98] fp16: J = c*128+p -> bin.
    j order within an ROI: (ysample, rowsel, xbin)."""
    pat = np.zeros((CH0, 128, NBIN2), np.float16)
    for J in range(2 * NS0):
        half, j = J // NS0, J % NS0
        bx = j % POOLED
        t = j // (2 * POOLED)
        py = t // 2
        pat[J // 128, J % 128, py * 7 + bx + half * NBIN] = 1.0
    return pat.transpose(1, 0, 2)                    # [128, CH0, 98]


def _strip_scatter(wd, w, half, ns, ngrp, width, t_base, chunks_w):
    """Scatter per-ROI strip weights [NROI_CORE, ns, width, 7] into wd
    [NPAIR, 128, NTD, 98]. half: 0/1 (roi parity within pair)."""
    jj = np.arange(ns)
    J = half * ns + jj
    ch, pp = J // 128, J % 128
    py = (jj // (2 * ngrp)) // 2
    t = t_base + ch[:, None] * chunks_w + np.arange(width)[None, :]  # [ns, width]
    bn = py[:, None] * 7 + np.arange(POOLED)[None, :] + half * NBIN  # [ns, 7]
    rois = np.arange(half, NROI_CORE, 2)
    pair_i = np.broadcast_to((rois // 2)[:, None, None, None],
                             (NPAIR, ns, width, POOLED))
    pp_i = np.broadcast_to(pp[None, :, None, None], pair_i.shape)
    t_i = np.broadcast_to(t[None, :, :, None], pair_i.shape)
    bn_i = np.broadcast_to(bn[None, :, None, :], pair_i.shape)
    np.add.at(wd, (pair_i, pp_i, t_i, bn_i), w[rois])


def _host_prepare(x0, x1, x2, x3, boxes):
    """Build all per-core input tensors. Returns list of 8 dicts."""
    B = boxes.shape[0]
    feats = []
    for arr, lv, rows in ((x0, L0, F0_ROWS), (x1, L1, F1_ROWS),
                          (x2, L2, F2_ROWS), (x3, L3, F3_ROWS)):
        f = np.zeros((B, rows, C), np.float16)
        hw = lv['H'] * lv['W']
        f[:, :hw] = np.ascontiguousarray(
            np.transpose(np.asarray(arr, np.float32), (0, 2, 3, 1))
        ).reshape(B, hw, C).astype(np.float16)
        feats.append(f)
    # f2 padded to [50, 64] rows so px = y*64+x -> y = 2ch + p//64, x = p%64
    f2pad = np.zeros((B, 50, 64, C), np.float16)
    f2pad[:, :, :50] = feats[2][:, :2500].reshape(B, 50, 50, C)
    f2s = np.ascontiguousarray(
        f2pad.reshape(B, CH2, 128, C).transpose(0, 2, 1, 3))
    # f3 padded to [25, 32] rows so px = y*32+x -> y = 4ch + p//32, x = p%32
    f3pad = np.zeros((B, 25, 32, C), np.float16)
    f3pad[:, :, :25] = feats[3][:, :625].reshape(B, 25, 25, C)
    f3s = np.ascontiguousarray(
        np.concatenate([f3pad.reshape(B, 800, C),
                        np.zeros((B, 96, C), np.float16)], axis=1)
        .reshape(B, CH3, 128, C).transpose(0, 2, 1, 3))

    pat0 = _l0_pattern()

    per_batch = []
    for b in range(B):
        bb = np.asarray(boxes[b], np.float32)
        m0 = _sample_meta(bb, L0['H'], L0['W'], L0['scale'])
        m1 = _sample_meta(bb, L1['H'], L1['W'], L1['scale'])
        idx0, w0 = _strip_grid(m0, L0['W'], 1, W0, parity=True)   # [N,196],[N,196,8,7]
        idx1, w1 = _strip_grid(m1, L1['W'], 2, W1, parity=False)  # [N,112],[N,112,9,7]
        per_batch.append((idx0, w0, idx1, w1))

    in_maps = []
    for k in range(8):
        b = k // 4
        s = (k % 4) * NROI_CORE
        idx0, w0, idx1, w1 = per_batch[b]
        sl = slice(s, s + NROI_CORE)
        bb = np.asarray(boxes[b][sl], np.float32)
        m2 = _sample_meta(bb, L2['H'], L2['W'], L2['scale'])
        m3 = _sample_meta(bb, L3['H'], L3['W'], L3['scale'])
        WY2t, WX2t = _sep_tables(m2, 50)           # [128, 50, 7] each
        WY3t, WX3t = _sep_tables(m3, 25)           # [128, 25, 7] each

        cst = np.zeros((128, CST_COLS), np.float16)
        cst[:, :PAT0_COLS] = pat0.reshape(128, -1)

        idxs = np.zeros((128, IDX_COLS), np.int16)
        wd = np.zeros((NPAIR, 128, NTD, NBIN2), np.float32)

        # dense lhsT: L1 strips tiles 0..17 (vectorized scatter)
        _strip_scatter(wd, w1[sl], 0, NS1, 4, W1, 0, W1)
        _strip_scatter(wd, w1[sl], 1, NS1, 4, W1, 0, W1)
        # separable tables per ROI:
        # L2: wy2 [128, 25, 7] (y = 2ch + p//64), wx2 [128, 7] (x = p%64)
        # L3: wy3 [128, 7, 7]  (y = 4ch + p//32), wx3 [128, 7] (x = p%32)
        pidx = np.arange(128)
        ych = (2 * np.arange(CH2)[None, :] + (pidx[:, None] // 64))  # [128,25]
        wy2 = WY2t[:, ych, :]                       # [128rois,128p,25,7]
        xp = pidx % 64
        wx2 = np.where((xp < 50)[None, :, None],
                       WX2t[:, np.minimum(xp, 49), :], 0.0)  # [128rois,128p,7]
        ych3 = (4 * np.arange(CH3)[None, :] + (pidx[:, None] // 32))  # [128,7]
        wy3 = np.where((ych3 < 25)[None, :, :, None],
                       WY3t[:, np.minimum(ych3, 24), :], 0.0)
        xp3 = pidx % 32
        wx3 = np.where((xp3 < 25)[None, :, None],
                       WX3t[:, np.minimum(xp3, 24), :], 0.0)
        wtab = np.concatenate(
            [wy2.reshape(NROI_CORE, 128, WY2), wx2,
             wy3.reshape(NROI_CORE, 128, WY3), wx3], axis=2)  # [128,128,238]
        wtab = wtab.reshape(NPAIR, 2, 128, TABH)

        for p in range(NPAIR):
            ra, rb = s + 2 * p, s + 2 * p + 1
            # --- L0: scal table + idx
            j0 = np.full(N0, -1, np.int64)
            j0[:NS0] = idx0[ra]
            j0[NS0:2 * NS0] = idx0[rb]
            sc = np.zeros((CH0 * 128, W0), np.float32)
            wpair = np.concatenate([w0[ra], w0[rb]], axis=0)  # [392, 8, 7]
            bins_x = np.tile(np.arange(NS0) % POOLED, 2)
            sc[:2 * NS0] = wpair[np.arange(2 * NS0), :, bins_x]
            cst[:, PAT0_COLS + p * SCAL0_PER_PAIR:
                PAT0_COLS + (p + 1) * SCAL0_PER_PAIR] = (
                sc.reshape(CH0, 128, W0).transpose(1, 0, 2)
                .reshape(128, SCAL0_PER_PAIR).astype(np.float16))
            # --- idx
            j1 = np.empty(N1, np.int64)
            j1[:NS1] = idx1[ra]
            j1[NS1:] = idx1[rb]
            col = p * PAIR_ICOLS
            idxs[:, col:col + N0 // 16] = _pack_idx(j0)
            col += N0 // 16
            idxs[:, col:col + N1 // 16] = _pack_idx(j1)

        wrow = np.concatenate(
            [wd.reshape(NPAIR, 128, NTD * NBIN2),
             wtab[:, 0], wtab[:, 1]], axis=2)       # [NPAIR, 128, 2618]
        in_maps.append({
            "f0": feats[0][b], "f1": feats[1][b],
            "f2s": f2s[b], "f3s": f3s[b],
            "cst": cst, "idxs": idxs,
            "wd": wrow.astype(np.float16).reshape(
                NBLK, 2, 128, PAIR_WROW).transpose(0, 2, 1, 3).reshape(
                NBLK, 128, 2 * PAIR_WROW),
        })
    return in_maps


def _build_module():
    from concourse import bacc, tile
    from concourse.bass import mybir
    import concourse.bass as bass_mod

    F32 = mybir.dt.float32
    F16 = mybir.dt.float16
    I16 = mybir.dt.int16
    AP = bass_mod.AP

    nc = bacc.Bacc(None, target_bir_lowering=False, num_swdge_queues=4)
    f0 = nc.dram_tensor("f0", [F0_ROWS, C], F16, kind="ExternalInput")
    f1 = nc.dram_tensor("f1", [F1_ROWS, C], F16, kind="ExternalInput")
    f2s = nc.dram_tensor("f2s", [128, CH2 * C], F16, kind="ExternalInput")
    f3s = nc.dram_tensor("f3s", [128, CH3 * C], F16, kind="ExternalInput")
    cst = nc.dram_tensor("cst", [128, CST_COLS], F16, kind="ExternalInput")
    idxs = nc.dram_tensor("idxs", [128, IDX_COLS], I16, kind="ExternalInput")
    wd = nc.dram_tensor("wd", [NBLK, 128, 2 * PAIR_WROW], F16,
                        kind="ExternalInput")
    out = nc.dram_tensor("out", [NBIN2, NPAIR, C], F16, kind="ExternalOutput")

    # strided views for strip gathers (strides/sizes in fp16 elements)
    f0v = AP(f0, 0, [[2 * C, (F0_ROWS - W0) // 2 + 1], [1, W0 * C]])
    f1v = AP(f1, 0, [[C, F1_ROWS - W1 + 1], [1, W1 * C]])

    # split gather calls (src, elem, step, chunk lo/hi, idx lo/hi, nidx, q)
    CALLS = [
        (0, f0v, W0 * C, 2 * C, 0, 2, 0, 16, 256, 0),
        (0, f0v, W0 * C, 2 * C, 2, 4, 16, 25, 144, 2),
        (1, f1v, W1 * C, C, 0, 1, 25, 33, 128, 1),
        (1, f1v, W1 * C, C, 1, 2, 33, 39, 96, 3),
    ]

    with tile.TileContext(nc) as tc:
        with (
            tc.tile_pool(name="const", bufs=1) as constp,
            tc.tile_pool(name="g0p", bufs=3) as g0p,
            tc.tile_pool(name="g1p", bufs=3) as g1p,
            tc.tile_pool(name="w0p", bufs=4) as w0p,
            tc.tile_pool(name="wdp", bufs=3) as wdp,
            tc.tile_pool(name="accp", bufs=8, space="PSUM") as accp,
            tc.tile_pool(name="evp", bufs=3) as evp,
        ):
            idx_t = constp.tile([128, IDX_COLS], I16)
            nc.sync.dma_start(idx_t[:], idxs[:])
            cst_t = constp.tile([128, CST_COLS], F16)
            nc.sync.dma_start(cst_t[:], cst[:])
            f2t = constp.tile([128, CH2, C], F16)
            nc.sync.dma_start(f2t[:], f2s.rearrange("p (h c) -> p h c", h=CH2))
            f3t = constp.tile([128, CH3, C], F16)
            nc.sync.dma_start(f3t[:], f3s.rearrange("p (h c) -> p h c", h=CH3))

            gpools = [g0p, g1p]
            pat0_ap = cst_t[:, 0:PAT0_COLS]
            n_mm = CH0 * W0 + NT1 + CH2 + CH3

            for blk in range(NBLK):
                wdt = wdp.tile([128, 2, PAIR_WROW], F16, tag="wd")
                nc.sync.dma_start(wdt[:], wd[blk].rearrange(
                    "p (i t) -> p i t", i=2))
                ev = evp.tile([NBIN2, 2, C], F16, tag="ev")
                for half in range(2):
                    p = blk * 2 + half
                    col = p * PAIR_ICOLS
                    gt0 = gpools[0].tile([128, CH0, W0 * C], F16, tag="g0")
                    gt1 = gpools[1].tile([128, CH1, W1 * C], F16, tag="g1")
                    gts = [gt0, gt1]
                    if p < 3:
                        nc.vector.memset(gts[0][:], 0)
                        nc.vector.memset(gts[1][:], 0)
                    for l, src, elem, step, c0, c1, i0, i1, nidx, q in CALLS:
                        # alternate queues by pair parity to balance bytes
                        qn = q ^ (2 if (p & 1) else 0)
                        nc.gpsimd.dma_gather(
                            gts[l][:, c0:c1, :], src,
                            idx_t[:, col + i0:col + i1],
                            nidx, nidx, elem, elem_step=step, queue_num=qn,
                            single_packet=False)

                    # L0 weights: one broadcast DVE op
                    w0t = w0p.tile([128, CH0, W0, NBIN2], F16, tag="w0")
                    pat_b = (pat0_ap.rearrange("p (c b) -> p c b", c=CH0)
                             .unsqueeze(2).broadcast_to((128, CH0, W0, NBIN2)))
                    so = PAT0_COLS + p * SCAL0_PER_PAIR
                    scal_b = (cst_t[:, so:so + SCAL0_PER_PAIR]
                              .rearrange("p (c s) -> p c s", c=CH0)
                              .unsqueeze(3).broadcast_to((128, CH0, W0, NBIN2)))
                    nc.vector.tensor_tensor(w0t[:], pat_b, scal_b,
                                            mybir.AluOpType.mult)
                    # L2/L3 weights: separable wy x wx, DVE ops per half
                    w2t = w0p.tile([128, CH2, NBIN2], F16, tag="w2")
                    w3t = w0p.tile([128, CH3, NBIN2], F16, tag="w3")
                    for h in range(2):
                        wyo = WYO + h * TABH
                        wy_b = (wdt[:, half, wyo:wyo + WY2]
                                .rearrange("p (c y) -> p c y", c=CH2)
                                .unsqueeze(3)
                                .broadcast_to((128, CH2, POOLED, POOLED)))
                        wx_b = (wdt[:, half, wyo + WY2:wyo + WY2 + POOLED]
                                .unsqueeze(1).unsqueeze(2)
                                .broadcast_to((128, CH2, POOLED, POOLED)))
                        nc.vector.tensor_tensor(
                            w2t[:, :, h * NBIN:(h + 1) * NBIN].rearrange(
                                "p c (a b) -> p c a b", a=POOLED),
                            wy_b, wx_b, mybir.AluOpType.mult)
                        o3 = wyo + WY2 + POOLED
                        wy3_b = (wdt[:, half, o3:o3 + WY3]
                                 .rearrange("p (c y) -> p c y", c=CH3)
                                 .unsqueeze(3)
                                 .broadcast_to((128, CH3, POOLED, POOLED)))
                        wx3_b = (wdt[:, half, o3 + WY3:o3 + WY3 + POOLED]
                                 .unsqueeze(1).unsqueeze(2)
                                 .broadcast_to((128, CH3, POOLED, POOLED)))
                        nc.vector.tensor_tensor(
                            w3t[:, :, h * NBIN:(h + 1) * NBIN].rearrange(
                                "p c (a b) -> p c a b", a=POOLED),
                            wy3_b, wx3_b, mybir.AluOpType.mult)

                    acc = accp.tile([NBIN2, C], F32)
                    mi = 0
                    for c in range(CH0):
                        for sl in range(W0):
                            nc.tensor.matmul(
                                acc[:], w0t[:, c, sl, :],
                                gts[0][:, c, sl * C:(sl + 1) * C],
                                start=(mi == 0), stop=(mi == n_mm - 1))
                            mi += 1
                    for c in range(CH1):
                        for sl in range(W1):
                            t = c * W1 + sl
                            nc.tensor.matmul(
                                acc[:], wdt[:, half, t * NBIN2:(t + 1) * NBIN2],
                                gts[1][:, c, sl * C:(sl + 1) * C],
                                start=(mi == 0), stop=(mi == n_mm - 1))
                            mi += 1
                    for c in range(CH2):
                        nc.tensor.matmul(
                            acc[:], w2t[:, c, :], f2t[:, c, :],
                            start=(mi == 0), stop=(mi == n_mm - 1))
                        mi += 1
                    for c in range(CH3):
                        nc.tensor.matmul(
                            acc[:], w3t[:, c, :], f3t[:, c, :],
                            start=(mi == 0), stop=(mi == n_mm - 1))
                        mi += 1

                    nc.scalar.copy(ev[:, half, :], acc[:])
                nc.sync.dma_start(out[:, 2 * blk:2 * blk + 2, :], ev[:])
    nc.finalize()
    return nc


def kernel(x0, x1, x2, x3, boxes):
    from concourse.bass_utils import run_bass_kernel_spmd
    in_maps = _host_prepare(x0, x1, x2, x3, boxes)
    if 'nc' not in _MODULE_CACHE:
        _MODULE_CACHE['nc'] = _build_module()
    nc = _MODULE_CACHE['nc']
    res = run_bass_kernel_spmd(nc, in_maps, list(range(8)))
    _MODULE_CACHE['last_result'] = res
    # per-core out is [98, 64, 256] bin-major: bin2 = half*49+bin
    parts = []
    for k in range(8):
        o = res.results[k]["out"].reshape(2, NBIN, NPAIR, C)
        parts.append(np.ascontiguousarray(
            o.transpose(2, 0, 3, 1)).reshape(NROI_CORE, C, NBIN))
    full = np.concatenate(parts, axis=0)           # [1024, 256, 49]
    return full.reshape(1024, C, POOLED, POOLED).astype(np.float32)


# revision 47
# speedup vs baseline: 1.1124x; 1.0015x over previous
"""Multi-level ROI Align (FPN pooler, 4 levels summed) on 8 Trainium2 cores.

v3.3: descriptor- and byte-minimized fp16 design. Shard ROIs across cores
(core k: batch k//4, 128 ROIs as 64 pairs). Host computes all gather
indices and bilinear weights from `boxes`; device does HBM strip-gathers
(dma_gather, one descriptor per multi-pixel fp16 strip, 4 SWDGE queues)
+ fp16 matmuls accumulating both ROIs of a pair into one PSUM tile
[98, 256] (bins 0-48 = ROI a, 49-97 = ROI b), evicted to DRAM bin-major
[98, 64pair, 256] fp32; host transposes.

Per pair-of-ROIs (62+ matmuls, ~616 gather descriptors):
  L0 (200x200, s=.25):  2x196 strips of 8px (even-aligned, idx=flat>>1),
      one per (ysample, rowsel, xbin); rank-1 weights (fixed one-hot bin
      pattern x per-strip scalar, one broadcast DVE op) -> 32 matmuls
  L1 (100x100, s=.125): 2x112 strips of 9px, one per (ysample, rowsel,
      xbin-pair); host-baked dense lhsT (DMA'd per block) -> 18 mm
  L2 (50x50, s=.0625):  NO gather -- full map SBUF-resident, padded to
      [50,64] rows (25 chunks); weights built on device from separable
      per-ROI wy[25ch,7]/wx[7] tables via 2 DVE ops -> 25 mm
  L3 (25x25, s=.03125): NO gather -- full map SBUF-resident (5 chunks);
      host-baked dense separable lhsT -> 5 mm
"""
import sys
import numpy as np

sys.path.insert(0, '/opt/trn_rl_repo')

POOLED = 7
SAMP = 2
NBIN = 49
C = 256
IMG = 800.0

NROI_CORE = 128
NPAIR = 64

# per-level geometry
L0 = dict(H=200, W=200, scale=0.25)
L1 = dict(H=100, W=100, scale=0.125)
L2 = dict(H=50, W=50, scale=0.0625)
L3 = dict(H=25, W=25, scale=0.03125)

W0, W1 = 8, 9                          # strip widths (px)
NS0, NS1 = 196, 112                    # strips per ROI
N0, N1 = 400, 224                      # padded pair nidx (%16, incl -1 tail)
CH0, CH1 = 4, 2                        # chunks per pair
CH2 = 25                               # L2 chunks: [50, 64]-padded map
CH3 = 7                                # L3 chunks: [25, 32]-padded map
NT1 = CH1 * W1                         # dense lhsT tiles (L1)
NTD = NT1                              # 18
NBIN2 = 2 * NBIN                       # 98
WY2 = CH2 * POOLED                     # 175 wy2 cols per ROI
WY3 = CH3 * POOLED                     # 49 wy3 cols per ROI
TABH = WY2 + POOLED + WY3 + POOLED     # 238 table els per ROI half
PAIR_WROW = NTD * NBIN2 + 2 * TABH     # 2240 wd els per pair
WYO = NTD * NBIN2                      # table offset in wd row
NBLK = NPAIR // 2                      # 2-pair blocks

F0_ROWS = 40004
F1_ROWS = 10000
F2_ROWS = 2560
F3_ROWS = 896

# cst fp16 column layout: L0 pattern [4ch, 98] then L0 scalars
PAT0_COLS = CH0 * NBIN2                        # 392
SCAL0_PER_PAIR = CH0 * W0                      # 32
CST_COLS = PAT0_COLS + NPAIR * SCAL0_PER_PAIR  # 392 + 2048

# idx int16 layout per pair: [L0 400][L1 224] / 16
PAIR_ICOLS = (N0 + N1) // 16                   # 39
IDX_COLS = NPAIR * PAIR_ICOLS

_MODULE_CACHE = {}


def _sample_meta(boxes_b, H, W, scale):
    """Per-ROI sample geometry in fp32, matching reference op order.
    boxes_b: [N, 4] fp32. Returns dict of [N,7,2] arrays."""
    f = np.float32
    b = boxes_b.astype(np.float32)
    x1 = b[:, 0] * f(scale)
    y1 = b[:, 1] * f(scale)
    x2 = b[:, 2] * f(scale)
    y2 = b[:, 3] * f(scale)
    rw = np.maximum(x2 - x1, f(1.0))
    rh = np.maximum(y2 - y1, f(1.0))
    bw = rw / f(POOLED)
    bh = rh / f(POOLED)
    g = (np.arange(POOLED, dtype=np.float32)[:, None]
         + (np.arange(SAMP, dtype=np.float32)[None, :] + f(0.5)) / f(SAMP))
    y = y1[:, None, None] + g[None] * bh[:, None, None]   # [N,7,2]
    x = x1[:, None, None] + g[None] * bw[:, None, None]
    masky = ((y >= f(-1.0)) & (y <= f(H))).astype(np.float32)
    maskx = ((x >= f(-1.0)) & (x <= f(W))).astype(np.float32)
    yc = np.clip(y, f(0.0), f(H - 1))
    xc = np.clip(x, f(0.0), f(W - 1))
    yl = np.floor(yc).astype(np.int64)
    xl = np.floor(xc).astype(np.int64)
    yh = np.minimum(yl + 1, H - 1)
    xh = np.minimum(xl + 1, W - 1)
    ly = (yc - yl.astype(np.float32)).astype(np.float32)
    lx = (xc - xl.astype(np.float32)).astype(np.float32)
    hy = (f(1.0) - ly).astype(np.float32)
    hx = (f(1.0) - lx).astype(np.float32)
    return dict(yl=yl, yh=yh, xl=xl, xh=xh, ly=ly, lx=lx, hy=hy, hx=hx,
                masky=masky, maskx=maskx, x=x, y=y)


def _strip_grid(meta, W, bins_per_strip, width, parity):
    """Build per-ROI strip indices and slot weights.

    Strips: (rowsel 2) x (ysample 14) x (xgroup ceil(7/b)).
    Returns idx [N, NS] int64 (pixel index of strip start, or start>>1 if
    parity), wslot [N, NS, width] fp32 (bilinear x-weights x y-weight x
    mask x 0.25), and bins [NS_xgroups arrays] for pattern building is
    implicit: each strip's samples' bins vary within the group -- handled
    by caller via per-sample info: also returns sample slot/bin arrays:
    contrib = (slotpos [N, NS, 7bins?..]) -- instead we return dense
    per-strip per-slot per-bin weights only when needed. For rank-1 (b=1)
    wslot is enough (all slots -> the strip's single bin).
    """
    N = meta['yl'].shape[0]
    f = np.float32
    ngrp = -(-POOLED // bins_per_strip)           # x-groups per row
    NS = 2 * 14 * ngrp
    # rows/yweights: [N, 2, 14]
    rows = np.stack([meta['yl'], meta['yh']], axis=1).reshape(N, 2, 14)
    wy = (np.stack([meta['hy'], meta['ly']], axis=1)
          * meta['masky'][:, None]).reshape(N, 2, 14).astype(np.float32)
    # x corners per sample: [N, 7, 2]
    xl, xh = meta['xl'], meta['xh']
    wxl = (meta['hx'] * meta['maskx']).astype(np.float32)
    wxh = (meta['lx'] * meta['maskx']).astype(np.float32)
    # group starts: min xl over samples in group -> [N, ngrp]
    xs = np.empty((N, ngrp), np.int64)
    for gi in range(ngrp):
        b0, b1 = gi * bins_per_strip, min((gi + 1) * bins_per_strip, POOLED)
        xs[:, gi] = xl[:, b0:b1, :].reshape(N, -1).min(axis=1)
    if parity:
        xs &= ~1
    xs = np.clip(xs, 0, W - width)
    # slot weights [N, ngrp, width] per (bin-in-group, sx): scatter
    wslot = np.zeros((N, ngrp, width, POOLED), np.float32)  # per-bin slots
    ridx = np.arange(N)[:, None, None]
    for gi in range(ngrp):
        b0, b1 = gi * bins_per_strip, min((gi + 1) * bins_per_strip, POOLED)
        for bx in range(b0, b1):
            for sx in range(SAMP):
                ol = xl[:, bx, sx] - xs[:, gi]
                oh = xh[:, bx, sx] - xs[:, gi]
                np.add.at(wslot, (np.arange(N), gi, ol, bx), wxl[:, bx, sx])
                np.add.at(wslot, (np.arange(N), gi, oh, bx), wxh[:, bx, sx])
    # combine with y: strips ordered (ysample, rowsel, xgroup) so the
    # yl/yh descriptors of one sample hit adjacent DRAM rows back-to-back
    idx = (rows[:, :, :, None] * W + xs[:, None, None, :])
    if parity:
        idx >>= 1
    idx = idx.transpose(0, 2, 1, 3).reshape(N, NS)       # [N,14,2,ngrp]
    w = (wy[:, :, :, None, None, None]
         * wslot[:, None, None, :, :, :] * f(0.25))      # [N,2,14,g,w,7]
    w = w.transpose(0, 2, 1, 3, 4, 5).reshape(N, NS, width, POOLED)
    return idx, w


def _sep_tables(meta, HW):
    """Separable bilinear weight tables WY/WX [N, HW, 7] fp32."""
    N = meta['yl'].shape[0]
    f = np.float32
    WY = np.zeros((N, HW, POOLED), np.float32)
    WX = np.zeros((N, HW, POOLED), np.float32)
    ridx = np.arange(N)[:, None, None]
    pidx = np.broadcast_to(np.arange(POOLED)[None, :, None], (N, POOLED, SAMP))
    np.add.at(WY, (ridx, meta['yl'], pidx),
              (f(0.5) * meta['hy'] * meta['masky']).astype(np.float32))
    np.add.at(WY, (ridx, meta['yh'], pidx),
              (f(0.5) * meta['ly'] * meta['masky']).astype(np.float32))
    np.add.at(WX, (ridx, meta['xl'], pidx),
              (f(0.5) * meta['hx'] * meta['maskx']).astype(np.float32))
    np.add.at(WX, (ridx, meta['xh'], pidx),
              (f(0.5) * meta['lx'] * meta['maskx']).astype(np.float32))
    return WY, WX


def _build_dense_full(meta, HW, nch):
    """Dense separable weights over the FULL HWxHW map (features live in
    SBUF on device). Returns lhsT [N, nch*128, 49] fp32 (px zero-padded)."""
    N = meta['yl'].shape[0]
    WY, WX = _sep_tables(meta, HW)
    lhsT = np.einsum('nap,nbq->nabpq', WY, WX).reshape(N, HW * HW, NBIN)
    out = np.zeros((N, nch * 128, NBIN), np.float32)
    out[:, :HW * HW] = lhsT
    return out


def _pack_idx(idx_flat):
    """[n] int -> [128, n//16] int16: wrap 16 partitions, replicate 8x."""
    n = idx_flat.shape[0]
    arr = idx_flat.reshape(n // 16, 16).T            # [16, cols]
    arr = np.broadcast_to(arr[None], (8, 16, n // 16)).reshape(128, n // 16)
    return arr.astype(np.int16)


def _l0_pattern():
    """Fixed one-hot [128, CH0, 98] fp16: J = c*128+p -> bin.
    j order within an ROI: (ysample, rowsel, xbin)."""
    pat = np.zeros((CH0, 128, NBIN2), np.float16)
    for J in range(2 * NS0):
        half, j = J // NS0, J % NS0
        bx = j % POOLED
        t = j // (2 * POOLED)
        py = t // 2
        pat[J // 128, J % 128, py * 7 + bx + half * NBIN] = 1.0
    return pat.transpose(1, 0, 2)                    # [128, CH0, 98]


def _strip_scatter(wd, w, half, ns, ngrp, width, t_base, chunks_w):
    """Scatter per-ROI strip weights [NROI_CORE, ns, width, 7] into wd
    [NPAIR, 128, NTD, 98]. half: 0/1 (roi parity within pair)."""
    jj = np.arange(ns)
    J = half * ns + jj
    ch, pp = J // 128, J % 128
    py = (jj // (2 * ngrp)) // 2
    t = t_base + ch[:, None] * chunks_w + np.arange(width)[None, :]  # [ns, width]
    bn = py[:, None] * 7 + np.arange(POOLED)[None, :] + half * NBIN  # [ns, 7]
    rois = np.arange(half, NROI_CORE, 2)
    pair_i = np.broadcast_to((rois // 2)[:, None, None, None],
                             (NPAIR, ns, width, POOLED))
    pp_i = np.broadcast_to(pp[None, :, None, None], pair_i.shape)
    t_i = np.broadcast_to(t[None, :, :, None], pair_i.shape)
    bn_i = np.broadcast_to(bn[None, :, None, :], pair_i.shape)
    np.add.at(wd, (pair_i, pp_i, t_i, bn_i), w[rois])


def _host_prepare(x0, x1, x2, x3, boxes):
    """Build all per-core input tensors. Returns list of 8 dicts."""
    B = boxes.shape[0]
    feats = []
    for arr, lv, rows in ((x0, L0, F0_ROWS), (x1, L1, F1_ROWS),
                          (x2, L2, F2_ROWS), (x3, L3, F3_ROWS)):
        f = np.zeros((B, rows, C), np.float16)
        hw = lv['H'] * lv['W']
        f[:, :hw] = np.ascontiguousarray(
            np.transpose(np.asarray(arr, np.float32), (0, 2, 3, 1))
        ).reshape(B, hw, C).astype(np.float16)
        feats.append(f)
    # f2 padded to [50, 64] rows so px = y*64+x -> y = 2ch + p//64, x = p%64
    f2pad = np.zeros((B, 50, 64, C), np.float16)
    f2pad[:, :, :50] = feats[2][:, :2500].reshape(B, 50, 50, C)
    f2s = np.ascontiguousarray(
        f2pad.reshape(B, CH2, 128, C).transpose(0, 2, 1, 3))
    # f3 padded to [25, 32] rows so px = y*32+x -> y = 4ch + p//32, x = p%32
    f3pad = np.zeros((B, 25, 32, C), np.float16)
    f3pad[:, :, :25] = feats[3][:, :625].reshape(B, 25, 25, C)
    f3s = np.ascontiguousarray(
        np.concatenate([f3pad.reshape(B, 800, C),
                        np.zeros((B, 96, C), np.float16)], axis=1)
        .reshape(B, CH3, 128, C).transpose(0, 2, 1, 3))

    pat0 = _l0_pattern()

    per_batch = []
    for b in range(B):
        bb = np.asarray(boxes[b], np.float32)
        m0 = _sample_meta(bb, L0['H'], L0['W'], L0['scale'])
        m1 = _sample_meta(bb, L1['H'], L1['W'], L1['scale'])
        idx0, w0 = _strip_grid(m0, L0['W'], 1, W0, parity=True)   # [N,196],[N,196,8,7]
        idx1, w1 = _strip_grid(m1, L1['W'], 2, W1, parity=False)  # [N,112],[N,112,9,7]
        per_batch.append((idx0, w0, idx1, w1))

    in_maps = []
    for k in range(8):
        b = k // 4
        s = (k % 4) * NROI_CORE
        idx0, w0, idx1, w1 = per_batch[b]
        sl = slice(s, s + NROI_CORE)
        bb = np.asarray(boxes[b][sl], np.float32)
        m2 = _sample_meta(bb, L2['H'], L2['W'], L2['scale'])
        m3 = _sample_meta(bb, L3['H'], L3['W'], L3['scale'])
        WY2t, WX2t = _sep_tables(m2, 50)           # [128, 50, 7] each
        WY3t, WX3t = _sep_tables(m3, 25)           # [128, 25, 7] each

        cst = np.zeros((128, CST_COLS), np.float16)
        cst[:, :PAT0_COLS] = pat0.reshape(128, -1)

        idxs = np.zeros((128, IDX_COLS), np.int16)
        wd = np.zeros((NPAIR, 128, NTD, NBIN2), np.float32)

        # dense lhsT: L1 strips tiles 0..17 (vectorized scatter)
        _strip_scatter(wd, w1[sl], 0, NS1, 4, W1, 0, W1)
        _strip_scatter(wd, w1[sl], 1, NS1, 4, W1, 0, W1)
        # separable tables per ROI:
        # L2: wy2 [128, 25, 7] (y = 2ch + p//64), wx2 [128, 7] (x = p%64)
        # L3: wy3 [128, 7, 7]  (y = 4ch + p//32), wx3 [128, 7] (x = p%32)
        pidx = np.arange(128)
        ych = (2 * np.arange(CH2)[None, :] + (pidx[:, None] // 64))  # [128,25]
        wy2 = WY2t[:, ych, :]                       # [128rois,128p,25,7]
        xp = pidx % 64
        wx2 = np.where((xp < 50)[None, :, None],
                       WX2t[:, np.minimum(xp, 49), :], 0.0)  # [128rois,128p,7]
        ych3 = (4 * np.arange(CH3)[None, :] + (pidx[:, None] // 32))  # [128,7]
        wy3 = np.where((ych3 < 25)[None, :, :, None],
                       WY3t[:, np.minimum(ych3, 24), :], 0.0)
        xp3 = pidx % 32
        wx3 = np.where((xp3 < 25)[None, :, None],
                       WX3t[:, np.minimum(xp3, 24), :], 0.0)
        wtab = np.concatenate(
            [wy2.reshape(NROI_CORE, 128, WY2), wx2,
             wy3.reshape(NROI_CORE, 128, WY3), wx3], axis=2)  # [128,128,238]
        wtab = wtab.reshape(NPAIR, 2, 128, TABH)

        for p in range(NPAIR):
            ra, rb = s + 2 * p, s + 2 * p + 1
            # --- L0: scal table + idx
            j0 = np.full(N0, -1, np.int64)
            j0[:NS0] = idx0[ra]
            j0[NS0:2 * NS0] = idx0[rb]
            sc = np.zeros((CH0 * 128, W0), np.float32)
            wpair = np.concatenate([w0[ra], w0[rb]], axis=0)  # [392, 8, 7]
            bins_x = np.tile(np.arange(NS0) % POOLED, 2)
            sc[:2 * NS0] = wpair[np.arange(2 * NS0), :, bins_x]
            cst[:, PAT0_COLS + p * SCAL0_PER_PAIR:
                PAT0_COLS + (p + 1) * SCAL0_PER_PAIR] = (
                sc.reshape(CH0, 128, W0).transpose(1, 0, 2)
                .reshape(128, SCAL0_PER_PAIR).astype(np.float16))
            # --- idx
            j1 = np.empty(N1, np.int64)
            j1[:NS1] = idx1[ra]
            j1[NS1:] = idx1[rb]
            col = p * PAIR_ICOLS
            idxs[:, col:col + N0 // 16] = _pack_idx(j0)
            col += N0 // 16
            idxs[:, col:col + N1 // 16] = _pack_idx(j1)

        wrow = np.concatenate(
            [wd.reshape(NPAIR, 128, NTD * NBIN2),
             wtab[:, 0], wtab[:, 1]], axis=2)       # [NPAIR, 128, 2618]
        in_maps.append({
            "f0": feats[0][b], "f1": feats[1][b],
            "f2s": f2s[b], "f3s": f3s[b],
            "cst": cst, "idxs": idxs,
            "wd": wrow.astype(np.float16).reshape(
                NBLK // 2, 4, 128, PAIR_WROW).transpose(0, 2, 1, 3).reshape(
                NBLK // 2, 128, 4 * PAIR_WROW),
        })
    return in_maps


def _build_module():
    from concourse import bacc, tile
    from concourse.bass import mybir
    import concourse.bass as bass_mod

    F32 = mybir.dt.float32
    F16 = mybir.dt.float16
    I16 = mybir.dt.int16
    AP = bass_mod.AP

    nc = bacc.Bacc(None, target_bir_lowering=False, num_swdge_queues=4)
    f0 = nc.dram_tensor("f0", [F0_ROWS, C], F16, kind="ExternalInput")
    f1 = nc.dram_tensor("f1", [F1_ROWS, C], F16, kind="ExternalInput")
    f2s = nc.dram_tensor("f2s", [128, CH2 * C], F16, kind="ExternalInput")
    f3s = nc.dram_tensor("f3s", [128, CH3 * C], F16, kind="ExternalInput")
    cst = nc.dram_tensor("cst", [128, CST_COLS], F16, kind="ExternalInput")
    idxs = nc.dram_tensor("idxs", [128, IDX_COLS], I16, kind="ExternalInput")
    wd = nc.dram_tensor("wd", [NBLK // 2, 128, 4 * PAIR_WROW], F16,
                        kind="ExternalInput")
    out = nc.dram_tensor("out", [NBIN2, NPAIR, C], F16, kind="ExternalOutput")

    # strided views for strip gathers (strides/sizes in fp16 elements)
    f0v = AP(f0, 0, [[2 * C, (F0_ROWS - W0) // 2 + 1], [1, W0 * C]])
    f1v = AP(f1, 0, [[C, F1_ROWS - W1 + 1], [1, W1 * C]])

    # split gather calls (src, elem, step, chunk lo/hi, idx lo/hi, nidx, q)
    CALLS = [
        (0, f0v, W0 * C, 2 * C, 0, 2, 0, 16, 256, 0),
        (0, f0v, W0 * C, 2 * C, 2, 4, 16, 25, 144, 2),
        (1, f1v, W1 * C, C, 0, 1, 25, 33, 128, 1),
        (1, f1v, W1 * C, C, 1, 2, 33, 39, 96, 3),
    ]

    with tile.TileContext(nc) as tc:
        with (
            tc.tile_pool(name="const", bufs=1) as constp,
            tc.tile_pool(name="g0p", bufs=3) as g0p,
            tc.tile_pool(name="g1p", bufs=3) as g1p,
            tc.tile_pool(name="w0p", bufs=4) as w0p,
            tc.tile_pool(name="wdp", bufs=3) as wdp,
            tc.tile_pool(name="accp", bufs=8, space="PSUM") as accp,
            tc.tile_pool(name="evp", bufs=3) as evp,
        ):
            idx_t = constp.tile([128, IDX_COLS], I16)
            nc.sync.dma_start(idx_t[:], idxs[:])
            cst_t = constp.tile([128, CST_COLS], F16)
            nc.sync.dma_start(cst_t[:], cst[:])
            f2t = constp.tile([128, CH2, C], F16)
            nc.sync.dma_start(f2t[:], f2s.rearrange("p (h c) -> p h c", h=CH2))
            f3t = constp.tile([128, CH3, C], F16)
            nc.sync.dma_start(f3t[:], f3s.rearrange("p (h c) -> p h c", h=CH3))

            gpools = [g0p, g1p]
            pat0_ap = cst_t[:, 0:PAT0_COLS]
            n_mm = CH0 * W0 + NT1 + CH2 + CH3

            for blk in range(NBLK):
                if blk % 2 == 0:
                    wdt4 = wdp.tile([128, 4, PAIR_WROW], F16, tag="wd")
                    nc.sync.dma_start(wdt4[:], wd[blk // 2].rearrange(
                        "p (i t) -> p i t", i=4))
                wdt = wdt4[:, 2 * (blk % 2):2 * (blk % 2) + 2, :]
                ev = evp.tile([NBIN2, 2, C], F16, tag="ev")
                for half in range(2):
                    p = blk * 2 + half
                    col = p * PAIR_ICOLS
                    gt0 = gpools[0].tile([128, CH0, W0 * C], F16, tag="g0")
                    gt1 = gpools[1].tile([128, CH1, W1 * C], F16, tag="g1")
                    gts = [gt0, gt1]
                    if p < 3:
                        nc.vector.memset(gts[0][:], 0)
                        nc.vector.memset(gts[1][:], 0)
                    for l, src, elem, step, c0, c1, i0, i1, nidx, q in CALLS:
                        # alternate queues by pair parity to balance bytes
                        qn = q ^ (2 if (p & 1) else 0)
                        nc.gpsimd.dma_gather(
                            gts[l][:, c0:c1, :], src,
                            idx_t[:, col + i0:col + i1],
                            nidx, nidx, elem, elem_step=step, queue_num=qn,
                            single_packet=False)

                    # L0 weights: one broadcast DVE op
                    w0t = w0p.tile([128, CH0, W0, NBIN2], F16, tag="w0")
                    pat_b = (pat0_ap.rearrange("p (c b) -> p c b", c=CH0)
                             .unsqueeze(2).broadcast_to((128, CH0, W0, NBIN2)))
                    so = PAT0_COLS + p * SCAL0_PER_PAIR
                    scal_b = (cst_t[:, so:so + SCAL0_PER_PAIR]
                              .rearrange("p (c s) -> p c s", c=CH0)
                              .unsqueeze(3).broadcast_to((128, CH0, W0, NBIN2)))
                    nc.vector.tensor_tensor(w0t[:], pat_b, scal_b,
                                            mybir.AluOpType.mult)
                    # L2/L3 weights: separable wy x wx, DVE ops per half
                    w2t = w0p.tile([128, CH2, NBIN2], F16, tag="w2")
                    w3t = w0p.tile([128, CH3, NBIN2], F16, tag="w3")
                    for h in range(2):
                        wyo = WYO + h * TABH
                        wy_b = (wdt[:, half, wyo:wyo + WY2]
                                .rearrange("p (c y) -> p c y", c=CH2)
                                .unsqueeze(3)
                                .broadcast_to((128, CH2, POOLED, POOLED)))
                        wx_b = (wdt[:, half, wyo + WY2:wyo + WY2 + POOLED]
                                .unsqueeze(1).unsqueeze(2)
                                .broadcast_to((128, CH2, POOLED, POOLED)))
                        nc.vector.tensor_tensor(
                            w2t[:, :, h * NBIN:(h + 1) * NBIN].rearrange(
                                "p c (a b) -> p c a b", a=POOLED),
                            wy_b, wx_b, mybir.AluOpType.mult)
                        o3 = wyo + WY2 + POOLED
                        wy3_b = (wdt[:, half, o3:o3 + WY3]
                                 .rearrange("p (c y) -> p c y", c=CH3)
                                 .unsqueeze(3)
                                 .broadcast_to((128, CH3, POOLED, POOLED)))
                        wx3_b = (wdt[:, half, o3 + WY3:o3 + WY3 + POOLED]
                                 .unsqueeze(1).unsqueeze(2)
                                 .broadcast_to((128, CH3, POOLED, POOLED)))
                        nc.vector.tensor_tensor(
                            w3t[:, :, h * NBIN:(h + 1) * NBIN].rearrange(
                                "p c (a b) -> p c a b", a=POOLED),
                            wy3_b, wx3_b, mybir.AluOpType.mult)

                    acc = accp.tile([NBIN2, C], F32)
                    mi = 0
                    for c in range(CH0):
                        for sl in range(W0):
                            nc.tensor.matmul(
                                acc[:], w0t[:, c, sl, :],
                                gts[0][:, c, sl * C:(sl + 1) * C],
                                start=(mi == 0), stop=(mi == n_mm - 1))
                            mi += 1
                    for c in range(CH1):
                        for sl in range(W1):
                            t = c * W1 + sl
                            nc.tensor.matmul(
                                acc[:], wdt[:, half, t * NBIN2:(t + 1) * NBIN2],
                                gts[1][:, c, sl * C:(sl + 1) * C],
                                start=(mi == 0), stop=(mi == n_mm - 1))
                            mi += 1
                    for c in range(CH2):
                        nc.tensor.matmul(
                            acc[:], w2t[:, c, :], f2t[:, c, :],
                            start=(mi == 0), stop=(mi == n_mm - 1))
                        mi += 1
                    for c in range(CH3):
                        nc.tensor.matmul(
                            acc[:], w3t[:, c, :], f3t[:, c, :],
                            start=(mi == 0), stop=(mi == n_mm - 1))
                        mi += 1

                    nc.scalar.copy(ev[:, half, :], acc[:])
                nc.sync.dma_start(out[:, 2 * blk:2 * blk + 2, :], ev[:])
    nc.finalize()
    return nc


def kernel(x0, x1, x2, x3, boxes):
    from concourse.bass_utils import run_bass_kernel_spmd
    in_maps = _host_prepare(x0, x1, x2, x3, boxes)
    if 'nc' not in _MODULE_CACHE:
        _MODULE_CACHE['nc'] = _build_module()
    nc = _MODULE_CACHE['nc']
    res = run_bass_kernel_spmd(nc, in_maps, list(range(8)))
    _MODULE_CACHE['last_result'] = res
    # per-core out is [98, 64, 256] bin-major: bin2 = half*49+bin
    parts = []
    for k in range(8):
        o = res.results[k]["out"].reshape(2, NBIN, NPAIR, C)
        parts.append(np.ascontiguousarray(
            o.transpose(2, 0, 3, 1)).reshape(NROI_CORE, C, NBIN))
    full = np.concatenate(parts, axis=0)           # [1024, 256, 49]
    return full.reshape(1024, C, POOLED, POOLED).astype(np.float32)


# revision 49
# speedup vs baseline: 1.1217x; 1.0084x over previous
"""Multi-level ROI Align (FPN pooler, 4 levels summed) on 8 Trainium2 cores.

v3.3: descriptor- and byte-minimized fp16 design. Shard ROIs across cores
(core k: batch k//4, 128 ROIs as 64 pairs). Host computes all gather
indices and bilinear weights from `boxes`; device does HBM strip-gathers
(dma_gather, one descriptor per multi-pixel fp16 strip, 4 SWDGE queues)
+ fp16 matmuls accumulating both ROIs of a pair into one PSUM tile
[98, 256] (bins 0-48 = ROI a, 49-97 = ROI b), evicted to DRAM bin-major
[98, 64pair, 256] fp32; host transposes.

Per pair-of-ROIs (62+ matmuls, ~616 gather descriptors):
  L0 (200x200, s=.25):  2x196 strips of 8px (even-aligned, idx=flat>>1),
      one per (ysample, rowsel, xbin); rank-1 weights (fixed one-hot bin
      pattern x per-strip scalar, one broadcast DVE op) -> 32 matmuls
  L1 (100x100, s=.125): 2x112 strips of 9px, one per (ysample, rowsel,
      xbin-pair); host-baked dense lhsT (DMA'd per block) -> 18 mm
  L2 (50x50, s=.0625):  NO gather -- full map SBUF-resident, padded to
      [50,64] rows (25 chunks); weights built on device from separable
      per-ROI wy[25ch,7]/wx[7] tables via 2 DVE ops -> 25 mm
  L3 (25x25, s=.03125): NO gather -- full map SBUF-resident (5 chunks);
      host-baked dense separable lhsT -> 5 mm
"""
import sys
import numpy as np

sys.path.insert(0, '/opt/trn_rl_repo')

POOLED = 7
SAMP = 2
NBIN = 49
C = 256
IMG = 800.0

NROI_CORE = 128
NPAIR = 64

# per-level geometry
L0 = dict(H=200, W=200, scale=0.25)
L1 = dict(H=100, W=100, scale=0.125)
L2 = dict(H=50, W=50, scale=0.0625)
L3 = dict(H=25, W=25, scale=0.03125)

W0, W1 = 8, 9                          # strip widths (px)
NS0, NS1 = 196, 112                    # strips per ROI
N0, N1 = 400, 224                      # padded pair nidx (%16, incl -1 tail)
CH0, CH1 = 4, 2                        # chunks per pair
CH2 = 25                               # L2 chunks: [50, 64]-padded map
CH3 = 7                                # L3 chunks: [25, 32]-padded map
NT1 = CH1 * W1                         # dense lhsT tiles (L1)
NTD = NT1                              # 18
NBIN2 = 2 * NBIN                       # 98
WY2 = CH2 * POOLED                     # 175 wy2 cols per ROI
WY3 = CH3 * POOLED                     # 49 wy3 cols per ROI
TABH = WY2 + POOLED + WY3 + POOLED     # 238 table els per ROI half
PAIR_WROW = NTD * NBIN2 + 2 * TABH     # 2240 wd els per pair
WYO = NTD * NBIN2                      # table offset in wd row
NBLK = NPAIR // 2                      # 2-pair blocks

F0_ROWS = 40004
F1_ROWS = 10000
F2_ROWS = 2560
F3_ROWS = 896

# cst fp16 column layout: L0 pattern [4ch, 98] then L0 scalars
PAT0_COLS = CH0 * NBIN2                        # 392
SCAL0_PER_PAIR = CH0 * W0                      # 32
CST_COLS = PAT0_COLS + NPAIR * SCAL0_PER_PAIR  # 392 + 2048

# idx int16 layout per pair: [L0 400][L1 224] / 16
PAIR_ICOLS = (N0 + N1) // 16                   # 39
IDX_COLS = NPAIR * PAIR_ICOLS

_MODULE_CACHE = {}


def _sample_meta(boxes_b, H, W, scale):
    """Per-ROI sample geometry in fp32, matching reference op order.
    boxes_b: [N, 4] fp32. Returns dict of [N,7,2] arrays."""
    f = np.float32
    b = boxes_b.astype(np.float32)
    x1 = b[:, 0] * f(scale)
    y1 = b[:, 1] * f(scale)
    x2 = b[:, 2] * f(scale)
    y2 = b[:, 3] * f(scale)
    rw = np.maximum(x2 - x1, f(1.0))
    rh = np.maximum(y2 - y1, f(1.0))
    bw = rw / f(POOLED)
    bh = rh / f(POOLED)
    g = (np.arange(POOLED, dtype=np.float32)[:, None]
         + (np.arange(SAMP, dtype=np.float32)[None, :] + f(0.5)) / f(SAMP))
    y = y1[:, None, None] + g[None] * bh[:, None, None]   # [N,7,2]
    x = x1[:, None, None] + g[None] * bw[:, None, None]
    masky = ((y >= f(-1.0)) & (y <= f(H))).astype(np.float32)
    maskx = ((x >= f(-1.0)) & (x <= f(W))).astype(np.float32)
    yc = np.clip(y, f(0.0), f(H - 1))
    xc = np.clip(x, f(0.0), f(W - 1))
    yl = np.floor(yc).astype(np.int64)
    xl = np.floor(xc).astype(np.int64)
    yh = np.minimum(yl + 1, H - 1)
    xh = np.minimum(xl + 1, W - 1)
    ly = (yc - yl.astype(np.float32)).astype(np.float32)
    lx = (xc - xl.astype(np.float32)).astype(np.float32)
    hy = (f(1.0) - ly).astype(np.float32)
    hx = (f(1.0) - lx).astype(np.float32)
    return dict(yl=yl, yh=yh, xl=xl, xh=xh, ly=ly, lx=lx, hy=hy, hx=hx,
                masky=masky, maskx=maskx, x=x, y=y)


def _strip_grid(meta, W, bins_per_strip, width, parity):
    """Build per-ROI strip indices and slot weights.

    Strips: (rowsel 2) x (ysample 14) x (xgroup ceil(7/b)).
    Returns idx [N, NS] int64 (pixel index of strip start, or start>>1 if
    parity), wslot [N, NS, width] fp32 (bilinear x-weights x y-weight x
    mask x 0.25), and bins [NS_xgroups arrays] for pattern building is
    implicit: each strip's samples' bins vary within the group -- handled
    by caller via per-sample info: also returns sample slot/bin arrays:
    contrib = (slotpos [N, NS, 7bins?..]) -- instead we return dense
    per-strip per-slot per-bin weights only when needed. For rank-1 (b=1)
    wslot is enough (all slots -> the strip's single bin).
    """
    N = meta['yl'].shape[0]
    f = np.float32
    ngrp = -(-POOLED // bins_per_strip)           # x-groups per row
    NS = 2 * 14 * ngrp
    # rows/yweights: [N, 2, 14]
    rows = np.stack([meta['yl'], meta['yh']], axis=1).reshape(N, 2, 14)
    wy = (np.stack([meta['hy'], meta['ly']], axis=1)
          * meta['masky'][:, None]).reshape(N, 2, 14).astype(np.float32)
    # x corners per sample: [N, 7, 2]
    xl, xh = meta['xl'], meta['xh']
    wxl = (meta['hx'] * meta['maskx']).astype(np.float32)
    wxh = (meta['lx'] * meta['maskx']).astype(np.float32)
    # group starts: min xl over samples in group -> [N, ngrp]
    xs = np.empty((N, ngrp), np.int64)
    for gi in range(ngrp):
        b0, b1 = gi * bins_per_strip, min((gi + 1) * bins_per_strip, POOLED)
        xs[:, gi] = xl[:, b0:b1, :].reshape(N, -1).min(axis=1)
    if parity:
        xs &= ~1
    xs = np.clip(xs, 0, W - width)
    # slot weights [N, ngrp, width] per (bin-in-group, sx): scatter
    wslot = np.zeros((N, ngrp, width, POOLED), np.float32)  # per-bin slots
    ridx = np.arange(N)[:, None, None]
    for gi in range(ngrp):
        b0, b1 = gi * bins_per_strip, min((gi + 1) * bins_per_strip, POOLED)
        for bx in range(b0, b1):
            for sx in range(SAMP):
                ol = xl[:, bx, sx] - xs[:, gi]
                oh = xh[:, bx, sx] - xs[:, gi]
                np.add.at(wslot, (np.arange(N), gi, ol, bx), wxl[:, bx, sx])
                np.add.at(wslot, (np.arange(N), gi, oh, bx), wxh[:, bx, sx])
    # combine with y: strips ordered (ysample, rowsel, xgroup) so the
    # yl/yh descriptors of one sample hit adjacent DRAM rows back-to-back
    idx = (rows[:, :, :, None] * W + xs[:, None, None, :])
    if parity:
        idx >>= 1
    idx = idx.transpose(0, 2, 1, 3).reshape(N, NS)       # [N,14,2,ngrp]
    w = (wy[:, :, :, None, None, None]
         * wslot[:, None, None, :, :, :] * f(0.25))      # [N,2,14,g,w,7]
    w = w.transpose(0, 2, 1, 3, 4, 5).reshape(N, NS, width, POOLED)
    return idx, w


def _sep_tables(meta, HW):
    """Separable bilinear weight tables WY/WX [N, HW, 7] fp32."""
    N = meta['yl'].shape[0]
    f = np.float32
    WY = np.zeros((N, HW, POOLED), np.float32)
    WX = np.zeros((N, HW, POOLED), np.float32)
    ridx = np.arange(N)[:, None, None]
    pidx = np.broadcast_to(np.arange(POOLED)[None, :, None], (N, POOLED, SAMP))
    np.add.at(WY, (ridx, meta['yl'], pidx),
              (f(0.5) * meta['hy'] * meta['masky']).astype(np.float32))
    np.add.at(WY, (ridx, meta['yh'], pidx),
              (f(0.5) * meta['ly'] * meta['masky']).astype(np.float32))
    np.add.at(WX, (ridx, meta['xl'], pidx),
              (f(0.5) * meta['hx'] * meta['maskx']).astype(np.float32))
    np.add.at(WX, (ridx, meta['xh'], pidx),
              (f(0.5) * meta['lx'] * meta['maskx']).astype(np.float32))
    return WY, WX


def _build_dense_full(meta, HW, nch):
    """Dense separable weights over the FULL HWxHW map (features live in
    SBUF on device). Returns lhsT [N, nch*128, 49] fp32 (px zero-padded)."""
    N = meta['yl'].shape[0]
    WY, WX = _sep_tables(meta, HW)
    lhsT = np.einsum('nap,nbq->nabpq', WY, WX).reshape(N, HW * HW, NBIN)
    out = np.zeros((N, nch * 128, NBIN), np.float32)
    out[:, :HW * HW] = lhsT
    return out


def _pack_idx(idx_flat):
    """[n] int -> [128, n//16] int16: wrap 16 partitions, replicate 8x."""
    n = idx_flat.shape[0]
    arr = idx_flat.reshape(n // 16, 16).T            # [16, cols]
    arr = np.broadcast_to(arr[None], (8, 16, n // 16)).reshape(128, n // 16)
    return arr.astype(np.int16)


def _l0_pattern():
    """Fixed one-hot [128, CH0, 98] fp16: J = c*128+p -> bin.
    j order within an ROI: (ysample, rowsel, xbin)."""
    pat = np.zeros((CH0, 128, NBIN2), np.float16)
    for J in range(2 * NS0):
        half, j = J // NS0, J % NS0
        bx = j % POOLED
        t = j // (2 * POOLED)
        py = t // 2
        pat[J // 128, J % 128, py * 7 + bx + half * NBIN] = 1.0
    return pat.transpose(1, 0, 2)                    # [128, CH0, 98]


def _strip_scatter(wd, w, half, ns, ngrp, width, t_base, chunks_w):
    """Scatter per-ROI strip weights [NROI_CORE, ns, width, 7] into wd
    [NPAIR, 128, NTD, 98]. half: 0/1 (roi parity within pair)."""
    jj = np.arange(ns)
    J = half * ns + jj
    ch, pp = J // 128, J % 128
    py = (jj // (2 * ngrp)) // 2
    t = t_base + ch[:, None] * chunks_w + np.arange(width)[None, :]  # [ns, width]
    bn = py[:, None] * 7 + np.arange(POOLED)[None, :] + half * NBIN  # [ns, 7]
    rois = np.arange(half, NROI_CORE, 2)
    pair_i = np.broadcast_to((rois // 2)[:, None, None, None],
                             (NPAIR, ns, width, POOLED))
    pp_i = np.broadcast_to(pp[None, :, None, None], pair_i.shape)
    t_i = np.broadcast_to(t[None, :, :, None], pair_i.shape)
    bn_i = np.broadcast_to(bn[None, :, None, :], pair_i.shape)
    np.add.at(wd, (pair_i, pp_i, t_i, bn_i), w[rois])


def _host_prepare(x0, x1, x2, x3, boxes):
    """Build all per-core input tensors. Returns list of 8 dicts."""
    B = boxes.shape[0]
    feats = []
    for arr, lv, rows in ((x0, L0, F0_ROWS), (x1, L1, F1_ROWS),
                          (x2, L2, F2_ROWS), (x3, L3, F3_ROWS)):
        f = np.zeros((B, rows, C), np.float16)
        hw = lv['H'] * lv['W']
        f[:, :hw] = np.ascontiguousarray(
            np.transpose(np.asarray(arr, np.float32), (0, 2, 3, 1))
        ).reshape(B, hw, C).astype(np.float16)
        feats.append(f)
    # f2 padded to [50, 64] rows so px = y*64+x -> y = 2ch + p//64, x = p%64
    f2pad = np.zeros((B, 50, 64, C), np.float16)
    f2pad[:, :, :50] = feats[2][:, :2500].reshape(B, 50, 50, C)
    f2s = np.ascontiguousarray(
        f2pad.reshape(B, CH2, 128, C).transpose(0, 2, 1, 3))
    # f3 padded to [25, 32] rows so px = y*32+x -> y = 4ch + p//32, x = p%32
    f3pad = np.zeros((B, 25, 32, C), np.float16)
    f3pad[:, :, :25] = feats[3][:, :625].reshape(B, 25, 25, C)
    f3s = np.ascontiguousarray(
        np.concatenate([f3pad.reshape(B, 800, C),
                        np.zeros((B, 96, C), np.float16)], axis=1)
        .reshape(B, CH3, 128, C).transpose(0, 2, 1, 3))

    pat0 = _l0_pattern()

    per_batch = []
    for b in range(B):
        bb = np.asarray(boxes[b], np.float32)
        m0 = _sample_meta(bb, L0['H'], L0['W'], L0['scale'])
        m1 = _sample_meta(bb, L1['H'], L1['W'], L1['scale'])
        idx0, w0 = _strip_grid(m0, L0['W'], 1, W0, parity=True)   # [N,196],[N,196,8,7]
        idx1, w1 = _strip_grid(m1, L1['W'], 2, W1, parity=False)  # [N,112],[N,112,9,7]
        per_batch.append((idx0, w0, idx1, w1))

    in_maps = []
    for k in range(8):
        b = k // 4
        s = (k % 4) * NROI_CORE
        idx0, w0, idx1, w1 = per_batch[b]
        sl = slice(s, s + NROI_CORE)
        bb = np.asarray(boxes[b][sl], np.float32)
        m2 = _sample_meta(bb, L2['H'], L2['W'], L2['scale'])
        m3 = _sample_meta(bb, L3['H'], L3['W'], L3['scale'])
        WY2t, WX2t = _sep_tables(m2, 50)           # [128, 50, 7] each
        WY3t, WX3t = _sep_tables(m3, 25)           # [128, 25, 7] each

        cst = np.zeros((128, CST_COLS), np.float16)
        cst[:, :PAT0_COLS] = pat0.reshape(128, -1)

        idxs = np.zeros((128, IDX_COLS), np.int16)
        wd = np.zeros((NPAIR, 128, NTD, NBIN2), np.float32)

        # dense lhsT: L1 strips tiles 0..17 (vectorized scatter)
        _strip_scatter(wd, w1[sl], 0, NS1, 4, W1, 0, W1)
        _strip_scatter(wd, w1[sl], 1, NS1, 4, W1, 0, W1)
        # separable tables per ROI:
        # L2: wy2 [128, 25, 7] (y = 2ch + p//64), wx2 [128, 7] (x = p%64)
        # L3: wy3 [128, 7, 7]  (y = 4ch + p//32), wx3 [128, 7] (x = p%32)
        pidx = np.arange(128)
        ych = (2 * np.arange(CH2)[None, :] + (pidx[:, None] // 64))  # [128,25]
        wy2 = WY2t[:, ych, :]                       # [128rois,128p,25,7]
        xp = pidx % 64
        wx2 = np.where((xp < 50)[None, :, None],
                       WX2t[:, np.minimum(xp, 49), :], 0.0)  # [128rois,128p,7]
        ych3 = (4 * np.arange(CH3)[None, :] + (pidx[:, None] // 32))  # [128,7]
        wy3 = np.where((ych3 < 25)[None, :, :, None],
                       WY3t[:, np.minimum(ych3, 24), :], 0.0)
        xp3 = pidx % 32
        wx3 = np.where((xp3 < 25)[None, :, None],
                       WX3t[:, np.minimum(xp3, 24), :], 0.0)
        wtab = np.concatenate(
            [wy2.reshape(NROI_CORE, 128, WY2), wx2,
             wy3.reshape(NROI_CORE, 128, WY3), wx3], axis=2)  # [128,128,238]
        wtab = wtab.reshape(NPAIR, 2, 128, TABH)

        for p in range(NPAIR):
            ra, rb = s + 2 * p, s + 2 * p + 1
            # --- L0: scal table + idx
            j0 = np.full(N0, -1, np.int64)
            j0[:NS0] = idx0[ra]
            j0[NS0:2 * NS0] = idx0[rb]
            sc = np.zeros((CH0 * 128, W0), np.float32)
            wpair = np.concatenate([w0[ra], w0[rb]], axis=0)  # [392, 8, 7]
            bins_x = np.tile(np.arange(NS0) % POOLED, 2)
            sc[:2 * NS0] = wpair[np.arange(2 * NS0), :, bins_x]
            cst[:, PAT0_COLS + p * SCAL0_PER_PAIR:
                PAT0_COLS + (p + 1) * SCAL0_PER_PAIR] = (
                sc.reshape(CH0, 128, W0).transpose(1, 0, 2)
                .reshape(128, SCAL0_PER_PAIR).astype(np.float16))
            # --- idx
            j1 = np.empty(N1, np.int64)
            j1[:NS1] = idx1[ra]
            j1[NS1:] = idx1[rb]
            col = p * PAIR_ICOLS
            idxs[:, col:col + N0 // 16] = _pack_idx(j0)
            col += N0 // 16
            idxs[:, col:col + N1 // 16] = _pack_idx(j1)

        wrow = np.concatenate(
            [wd.reshape(NPAIR, 128, NTD * NBIN2),
             wtab[:, 0], wtab[:, 1]], axis=2)       # [NPAIR, 128, 2618]
        in_maps.append({
            "f0": feats[0][b], "f1": feats[1][b],
            "f2s": f2s[b], "f3s": f3s[b],
            "cst": cst, "idxs": idxs,
            "wd": wrow.astype(np.float16).reshape(
                NBLK // 2, 4, 128, PAIR_WROW).transpose(0, 2, 1, 3).reshape(
                NBLK // 2, 128, 4 * PAIR_WROW),
        })
    return in_maps


def _build_module():
    from concourse import bacc, tile
    from concourse.bass import mybir
    import concourse.bass as bass_mod

    F32 = mybir.dt.float32
    F16 = mybir.dt.float16
    I16 = mybir.dt.int16
    AP = bass_mod.AP

    nc = bacc.Bacc(None, target_bir_lowering=False, num_swdge_queues=4)
    f0 = nc.dram_tensor("f0", [F0_ROWS, C], F16, kind="ExternalInput")
    f1 = nc.dram_tensor("f1", [F1_ROWS, C], F16, kind="ExternalInput")
    f2s = nc.dram_tensor("f2s", [128, CH2 * C], F16, kind="ExternalInput")
    f3s = nc.dram_tensor("f3s", [128, CH3 * C], F16, kind="ExternalInput")
    cst = nc.dram_tensor("cst", [128, CST_COLS], F16, kind="ExternalInput")
    idxs = nc.dram_tensor("idxs", [128, IDX_COLS], I16, kind="ExternalInput")
    wd = nc.dram_tensor("wd", [NBLK // 2, 128, 4 * PAIR_WROW], F16,
                        kind="ExternalInput")
    out = nc.dram_tensor("out", [NBIN2, NPAIR, C], F16, kind="ExternalOutput")

    # strided views for strip gathers (strides/sizes in fp16 elements)
    f0v = AP(f0, 0, [[2 * C, (F0_ROWS - W0) // 2 + 1], [1, W0 * C]])
    f1v = AP(f1, 0, [[C, F1_ROWS - W1 + 1], [1, W1 * C]])

    # split gather calls (src, elem, step, chunk lo/hi, idx lo/hi, nidx, q)
    CALLS = [
        (0, f0v, W0 * C, 2 * C, 0, 2, 0, 16, 256, 0),
        (0, f0v, W0 * C, 2 * C, 2, 4, 16, 25, 144, 2),
        (1, f1v, W1 * C, C, 0, 1, 25, 33, 128, 1),
        (1, f1v, W1 * C, C, 1, 2, 33, 39, 96, 3),
    ]

    with tile.TileContext(nc) as tc:
        with (
            tc.tile_pool(name="const", bufs=1) as constp,
            tc.tile_pool(name="g0p", bufs=3) as g0p,
            tc.tile_pool(name="g1p", bufs=3) as g1p,
            tc.tile_pool(name="w0p", bufs=4) as w0p,
            tc.tile_pool(name="wdp", bufs=3) as wdp,
            tc.tile_pool(name="accp", bufs=8, space="PSUM") as accp,
            tc.tile_pool(name="evp", bufs=3) as evp,
        ):
            idx_t = constp.tile([128, IDX_COLS], I16)
            nc.sync.dma_start(idx_t[:], idxs[:])
            cst_t = constp.tile([128, CST_COLS], F16)
            nc.sync.dma_start(cst_t[:], cst[:])
            f2t = constp.tile([128, CH2, C], F16)
            nc.sync.dma_start(f2t[:], f2s.rearrange("p (h c) -> p h c", h=CH2))
            f3t = constp.tile([128, CH3, C], F16)
            nc.sync.dma_start(f3t[:], f3s.rearrange("p (h c) -> p h c", h=CH3))

            gpools = [g0p, g1p]
            pat0_ap = cst_t[:, 0:PAT0_COLS]
            n_mm = CH0 * W0 + NT1 + CH2 + CH3

            for blk in range(NBLK):
                if blk % 2 == 0:
                    wdt4 = wdp.tile([128, 4, PAIR_WROW], F16, tag="wd")
                    nc.sync.dma_start(wdt4[:], wd[blk // 2].rearrange(
                        "p (i t) -> p i t", i=4))
                wdt = wdt4[:, 2 * (blk % 2):2 * (blk % 2) + 2, :]
                ev = evp.tile([NBIN2, 2, C], F16, tag="ev")
                for half in range(2):
                    p = blk * 2 + half
                    col = p * PAIR_ICOLS
                    gt0 = gpools[0].tile([128, CH0, W0 * C], F16, tag="g0")
                    gt1 = gpools[1].tile([128, CH1, W1 * C], F16, tag="g1")
                    gts = [gt0, gt1]
                    if p < 3:
                        nc.vector.memset(gts[0][:], 0)
                        nc.vector.memset(gts[1][:], 0)
                    for l, src, elem, step, c0, c1, i0, i1, nidx, q in CALLS:
                        # alternate queues by pair parity to balance bytes
                        qn = q ^ (2 if (p & 1) else 0)
                        nc.gpsimd.dma_gather(
                            gts[l][:, c0:c1, :], src,
                            idx_t[:, col + i0:col + i1],
                            nidx, nidx, elem, elem_step=step, queue_num=qn,
                            single_packet=False)

                    # L0 weights: one broadcast DVE op
                    w0t = w0p.tile([128, CH0, W0, NBIN2], F16, tag="w0")
                    pat_b = (pat0_ap.rearrange("p (c b) -> p c b", c=CH0)
                             .unsqueeze(2).broadcast_to((128, CH0, W0, NBIN2)))
                    so = PAT0_COLS + p * SCAL0_PER_PAIR
                    scal_b = (cst_t[:, so:so + SCAL0_PER_PAIR]
                              .rearrange("p (c s) -> p c s", c=CH0)
                              .unsqueeze(3).broadcast_to((128, CH0, W0, NBIN2)))
                    nc.vector.tensor_tensor(w0t[:], pat_b, scal_b,
                                            mybir.AluOpType.mult)
                    # L2/L3 weights: separable wy x wx, DVE ops per half
                    w2t = w0p.tile([128, CH2, NBIN2], F16, tag="w2")
                    w3t = w0p.tile([128, CH3, NBIN2], F16, tag="w3")
                    for h in range(2):
                        wyo = WYO + h * TABH
                        wy_b = (wdt[:, half, wyo:wyo + WY2]
                                .rearrange("p (c y) -> p c y", c=CH2)
                                .unsqueeze(3)
                                .broadcast_to((128, CH2, POOLED, POOLED)))
                        wx_b = (wdt[:, half, wyo + WY2:wyo + WY2 + POOLED]
                                .unsqueeze(1).unsqueeze(2)
                                .broadcast_to((128, CH2, POOLED, POOLED)))
                        nc.vector.tensor_tensor(
                            w2t[:, :, h * NBIN:(h + 1) * NBIN].rearrange(
                                "p c (a b) -> p c a b", a=POOLED),
                            wy_b, wx_b, mybir.AluOpType.mult)
                        o3 = wyo + WY2 + POOLED
                        wy3_b = (wdt[:, half, o3:o3 + WY3]
                                 .rearrange("p (c y) -> p c y", c=CH3)
                                 .unsqueeze(3)
                                 .broadcast_to((128, CH3, POOLED, POOLED)))
                        wx3_b = (wdt[:, half, o3 + WY3:o3 + WY3 + POOLED]
                                 .unsqueeze(1).unsqueeze(2)
                                 .broadcast_to((128, CH3, POOLED, POOLED)))
                        nc.vector.tensor_tensor(
                            w3t[:, :, h * NBIN:(h + 1) * NBIN].rearrange(
                                "p c (a b) -> p c a b", a=POOLED),
                            wy3_b, wx3_b, mybir.AluOpType.mult)

                    acc = accp.tile([NBIN2, C], F32)
                    mi = 0
                    for c in range(CH0):
                        for sl in range(W0):
                            nc.tensor.matmul(
                                acc[:], w0t[:, c, sl, :],
                                gts[0][:, c, sl * C:(sl + 1) * C],
                                start=(mi == 0), stop=(mi == n_mm - 1))
                            mi += 1
                    for c in range(CH1):
                        for sl in range(W1):
                            t = c * W1 + sl
                            nc.tensor.matmul(
                                acc[:], wdt[:, half, t * NBIN2:(t + 1) * NBIN2],
                                gts[1][:, c, sl * C:(sl + 1) * C],
                                start=(mi == 0), stop=(mi == n_mm - 1))
                            mi += 1
                    for c in range(CH2):
                        nc.tensor.matmul(
                            acc[:], w2t[:, c, :], f2t[:, c, :],
                            start=(mi == 0), stop=(mi == n_mm - 1))
                        mi += 1
                    for c in range(CH3):
                        nc.tensor.matmul(
                            acc[:], w3t[:, c, :], f3t[:, c, :],
                            start=(mi == 0), stop=(mi == n_mm - 1))
                        mi += 1

                    nc.scalar.copy(ev[:, half, :], acc[:])
                nc.sync.dma_start(out[:, 2 * blk:2 * blk + 2, :], ev[:])
    nc.finalize()
    return nc


def kernel(x0, x1, x2, x3, boxes):
    from concourse.bass_utils import run_bass_kernel_spmd
    in_maps = _host_prepare(x0, x1, x2, x3, boxes)
    if 'nc' not in _MODULE_CACHE:
        _MODULE_CACHE['nc'] = _build_module()
    nc = _MODULE_CACHE['nc']
    res = run_bass_kernel_spmd(nc, in_maps, list(range(8)))
    _MODULE_CACHE['last_result'] = res
    # per-core out is [98, 64, 256] bin-major: bin2 = half*49+bin
    parts = []
    for k in range(8):
        o = res.results[k]["out"].reshape(2, NBIN, NPAIR, C)
        parts.append(np.ascontiguousarray(
            o.transpose(2, 0, 3, 1)).reshape(NROI_CORE, C, NBIN))
    full = np.concatenate(parts, axis=0)           # [1024, 256, 49]
    return full.reshape(1024, C, POOLED, POOLED).astype(np.float32)
